# revision 7
# baseline (speedup 1.0000x reference)
"""Trainium2 Bass kernel for nn_ExpectedSignature.

Computes, for signatures x[B=64, S=32, L=19530] (L = sum_{k=1..6} 5^k):
  1. per-(b,s) level sums  l_k = sum_{i in level k} x_i^2
  2. c0 = 1 - phi(1 + sum_k l_k)   (phi(x) = x for x<=4 else 8 - 16/x)
  3. root u of  c0 + sum_k l_k u^k = 0  on [0,1]  (u = t^2, t = dilatation norm)
  4. out[b, i] = mean_s x[b,s,i] * t^{level(i)}

Sharding: data-parallel over batch, 8 batches per core on 8 cores.

Per-core layout: x rows (b_local*32 + s) -> 2 partition groups of 128 rows
(4 batches x 32 samples each).  Phase 1 streams x from HBM into resident
SBUF tiles while computing level sums with fused square+reduce ops split
across the Vector and Scalar engines.  The root solve is a Newton
iteration on [128,1] tiles (seeded by exp(log(-c0/l6)/6)).  Phase 2 fuses
the t^level scaling and the sample-mean into tensor-engine matmuls whose
stationary weights are (batch-onehot/32) * t^level, reading the resident
SBUF x tiles; PSUM results DMA straight to HBM.
"""

import math
import os
from contextlib import ExitStack

import numpy as np

import concourse.bass as bass
import concourse.bacc as bacc
import concourse.mybir as mybir
import concourse.tile as tile
from concourse import bass_utils

F32 = mybir.dt.float32
F32R = mybir.dt.float32r
AF = mybir.ActivationFunctionType
ALU = mybir.AluOpType
AX = mybir.AxisListType

B, S, L = 64, 32, 19530
N_CORES = 8
B_LOC = B // N_CORES          # 8 batches per core
ROWS = B_LOC * S              # 256 rows per core
N_GROUPS = 2                  # 2 partition groups of 128 rows
BPG = 4                       # batches per group
LEVEL_STARTS = [0, 5, 30, 155, 780, 3905, 19530]

CONFIG = {
    "n_newton": 6,
    "chunk": 1302,            # phase-1 compute/DMA chunk (columns)
    "mm_dtype": "f32",        # "f32" | "f32r"
    "sqrt_refine": 2,         # Newton refinements after ScalarE sqrt
    "psum_cols": 1024,        # columns per PSUM output tile (multiple of 512)
    "scalar_share": True,     # split phase-1 between Vector and Scalar engines
}

_cache = {}


def _chunk_plan(chunk):
    """Per level, split [start, end) into pieces <= chunk.
    Returns list of (level_idx, col0, col1)."""
    plan = []
    for k in range(6):
        c0, c1 = LEVEL_STARTS[k], LEVEL_STARTS[k + 1]
        n = c1 - c0
        pieces = max(1, math.ceil(n / chunk))
        base = n // pieces
        rem = n % pieces
        a = c0
        for p in range(pieces):
            sz = base + (1 if p < rem else 0)
            plan.append((k, a, a + sz))
            a += sz
        assert a == c1
    return plan


def _assign_engines(plan, scalar_share):
    """Engine per chunk: 'v' (vector) or 's' (scalar)."""
    eng = []
    flip = 0
    for (k, a, b) in plan:
        if not scalar_share:
            eng.append("v")
        elif k <= 2:              # tiny levels: vector (lower per-op overhead)
            eng.append("v")
        elif k == 3:
            eng.append("s")
        else:                     # big levels: alternate
            eng.append("s" if flip % 2 == 0 else "v")
            flip += 1
    return eng


def _mm_segments(psum_cols):
    """Output column segments for matmuls: split L at level boundaries and
    the 512-column PSUM-bank grid; grouped into PSUM tiles of psum_cols."""
    bounds = set(LEVEL_STARTS)
    bounds.update(range(0, L + 1, 512))
    bounds.add(L)
    bounds = sorted(b for b in bounds if 0 <= b <= L)
    segs = []
    for a, b in zip(bounds[:-1], bounds[1:]):
        # level of this segment
        k = next(i for i in range(6) if LEVEL_STARTS[i] <= a < LEVEL_STARTS[i + 1])
        assert b <= LEVEL_STARTS[k + 1]
        segs.append((k, a, b))
    # group into psum tiles
    tiles = []
    for p0 in range(0, L, psum_cols):
        p1 = min(p0 + psum_cols, L)
        tiles.append((p0, p1, [s for s in segs if p0 <= s[1] < p1 or (p0 <= s[1] and s[2] <= p1)]))
    # simpler: recompute membership
    tiles = []
    for p0 in range(0, L, psum_cols):
        p1 = min(p0 + psum_cols, L)
        members = [s for s in segs if s[1] >= p0 and s[2] <= p1]
        assert sum(s[2] - s[1] for s in members) == p1 - p0
        tiles.append((p0, p1, members))
    return tiles


def _build_kernel(cfg):
    nc = bacc.Bacc(
        "TRN2",
        target_bir_lowering=False,
        debug=False,
        num_devices=N_CORES,
    )
    x = nc.dram_tensor("x", [ROWS, L], F32, kind="ExternalInput").ap()
    wsel = nc.dram_tensor("wsel", [128, BPG], F32, kind="ExternalInput").ap()
    out = nc.dram_tensor("out", [B_LOC, L], F32, kind="ExternalOutput").ap()

    plan = _chunk_plan(cfg["chunk"])
    engines = _assign_engines(plan, cfg["scalar_share"])
    n_chunks = len(plan)
    mm_tiles = _mm_segments(cfg["psum_cols"])
    mm_dt = F32R if cfg["mm_dtype"] == "f32r" else F32

    # chunk-index ranges per level (for per-level partial reduction)
    lvl_chunk_rng = []
    for k in range(6):
        idxs = [i for i, (kk, _, _) in enumerate(plan) if kk == k]
        lvl_chunk_rng.append((min(idxs), max(idxs) + 1))

    with ExitStack() as ctx:
        tc = ctx.enter_context(tile.TileContext(nc))
        xg_pool = ctx.enter_context(tc.tile_pool(name="xg", bufs=1))
        cst_pool = ctx.enter_context(tc.tile_pool(name="cst", bufs=1))
        scr_v = ctx.enter_context(tc.tile_pool(name="scr_v", bufs=2))
        scr_s = ctx.enter_context(tc.tile_pool(name="scr_s", bufs=2))
        sol_pool = ctx.enter_context(tc.tile_pool(name="sol", bufs=2))
        psum_pool = ctx.enter_context(tc.tile_pool(name="psum", bufs=2, space="PSUM"))

        # constants
        wsel_t = cst_pool.tile([128, BPG], F32, tag="wsel")
        nc.sync.dma_start(wsel_t[:], wsel)
        kmul = cst_pool.tile([128, 6], F32, tag="kmul")
        for j in range(6):
            nc.vector.memset(kmul[:, j:j + 1], float(j + 1))

        # persistent per-group tiles
        XG, PART, LVW, W = [], [], [], []
        for g in range(N_GROUPS):
            XG.append(xg_pool.tile([128, L], F32, name=f"xg{g}", tag=f"xg{g}"))
            PART.append(cst_pool.tile([128, n_chunks], F32, name=f"part{g}", tag=f"part{g}"))
            # LVW cols: 0..5 l_k | 6 c0 | 7..12 k*l_k | 13 zero
            LVW.append(cst_pool.tile([128, 14], F32, name=f"lvw{g}", tag=f"lvw{g}"))
            W.append(cst_pool.tile([128, 6 * BPG], F32, name=f"w{g}", tag=f"w{g}"))

        # ---- phase 1: DMA in + square + per-level partial sums ----
        for g in range(N_GROUPS):
            rows = slice(g * 128, (g + 1) * 128)
            for ci, (k, a, b) in enumerate(plan):
                xt = XG[g][:, a:b]
                nc.sync.dma_start(xt, x[rows, a:b])
                acc = PART[g][:, ci:ci + 1]
                if engines[ci] == "v":
                    scr = scr_v.tile([128, cfg["chunk"]], F32, name="scrv", tag="scr_v")
                    nc.vector.scalar_tensor_tensor(
                        out=scr[:, : b - a], in0=xt, scalar=1.0, in1=xt,
                        op0=ALU.bypass, op1=ALU.mult, accum_out=acc)
                else:
                    scr = scr_s.tile([128, cfg["chunk"]], F32, name="scrs", tag="scr_s")
                    nc.scalar.activation(
                        out=scr[:, : b - a], in_=xt, func=AF.Square,
                        accum_out=acc)

        # ---- solve + weights (per group) ----
        for g in range(N_GROUPS):
            lvw = LVW[g]
            for k in range(6):
                lo, hi = lvl_chunk_rng[k]
                if hi - lo == 1:
                    nc.vector.tensor_copy(lvw[:, k:k + 1], PART[g][:, lo:lo + 1])
                else:
                    nc.vector.tensor_reduce(
                        out=lvw[:, k:k + 1], in_=PART[g][:, lo:hi],
                        axis=AX.X, op=ALU.add)
            sl = sol_pool.tile([128, 16], F32, name="sl", tag="sl")
            sumlv = sl[:, 0:1]
            nq = sl[:, 1:2]
            rnq = sl[:, 2:3]
            c0b = sl[:, 3:4]
            c0s = sl[:, 4:5]
            msk = sl[:, 5:6]
            rl6 = sl[:, 6:7]
            t1 = sl[:, 7:8]
            lg = sl[:, 8:9]
            us = sl[:, 9:10]
            tt = sl[:, 10:11]
            rt = sl[:, 11:12]
            tmp = sl[:, 12:13]
            tt2 = sl[:, 13:14]

            nc.vector.tensor_reduce(out=sumlv, in_=lvw[:, 0:6], axis=AX.X, op=ALU.add)
            nc.vector.tensor_scalar(nq, sumlv, 1.0, None, ALU.add)
            nc.vector.reciprocal(rnq, nq)
            nc.vector.tensor_scalar(c0b, rnq, 16.0, -7.0, ALU.mult, ALU.add)
            nc.vector.tensor_scalar(c0s, nq, -1.0, 1.0, ALU.mult, ALU.add)
            nc.vector.tensor_scalar(msk, nq, 4.0, None, ALU.is_gt)
            nc.vector.tensor_sub(tmp, c0b, c0s)
            nc.vector.scalar_tensor_tensor(
                lvw[:, 6:7], tmp, msk[:, 0:1], c0s, op0=ALU.mult, op1=ALU.add)
            nc.vector.memset(lvw[:, 13:14], 0.0)
            nc.vector.tensor_tensor(lvw[:, 7:13], lvw[:, 0:6], kmul[:], ALU.mult)

            # u0 = min(1, exp(log(-c0/l6)/6))
            nc.vector.reciprocal(rl6, lvw[:, 5:6])
            nc.vector.scalar_tensor_tensor(
                t1, lvw[:, 6:7], -1.0, rl6, op0=ALU.mult, op1=ALU.mult)
            nc.scalar.activation(lg, t1, AF.Ln)
            nc.scalar.activation(us, lg, AF.Exp, scale=1.0 / 6.0)

            ping = sol_pool.tile([128, 1], F32, name=f"u{g}a", tag=f"u{g}a")
            pong = sol_pool.tile([128, 1], F32, name=f"u{g}b", tag=f"u{g}b")
            nc.vector.tensor_scalar_min(ping, us, 1.0)
            u, unext = ping, pong

            def pair(k):
                # columns (l_k, k*l_k); for k==0: (c0, 0)
                c = k - 1 if k >= 1 else 6
                return lvw[:, c:c + 8:7]

            pq = sol_pool.tile([128, 2], F32, name=f"pq{g}", tag=f"pq{g}")
            for it in range(cfg["n_newton"]):
                d = sl[:, 14:15]
                rq = sl[:, 15:16]
                nc.vector.scalar_tensor_tensor(
                    pq[:], pair(6), u[:, 0:1], pair(5), op0=ALU.mult, op1=ALU.add)
                for k in (4, 3, 2, 1, 0):
                    nc.vector.scalar_tensor_tensor(
                        pq[:], pq[:], u[:, 0:1], pair(k), op0=ALU.mult, op1=ALU.add)
                nc.vector.tensor_sub(d, pq[:, 1:2], pq[:, 0:1])
                nc.vector.reciprocal(rq, pq[:, 1:2])
                nc.vector.scalar_tensor_tensor(
                    unext[:], d, rq[:, 0:1], u[:], op0=ALU.mult, op1=ALU.mult)
                u, unext = unext, u

            # t = min(1, sqrt(u)) with Newton refinements
            nc.scalar.activation(tt, u[:], AF.Sqrt)
            tcur = tt
            for r in range(cfg["sqrt_refine"]):
                nc.vector.reciprocal(rt, tcur)
                nc.vector.scalar_tensor_tensor(
                    tmp, rt, u[:, 0:1], tcur, op0=ALU.mult, op1=ALU.add)
                nxt = tt2 if tcur is tt else tt
                nc.vector.tensor_scalar(nxt, tmp, 0.5, None, ALU.mult)
                tcur = nxt
            tfin = sl[:, 14:15] if cfg["sqrt_refine"] == 0 else tcur
            nc.vector.tensor_scalar_min(tfin, tcur, 1.0)

            # factors F = (t, u, ut, u2, u2t, u3)
            F = sol_pool.tile([128, 6], F32, name=f"fac{g}", tag=f"fac{g}")
            nc.vector.tensor_copy(F[:, 0:1], tfin)
            nc.vector.tensor_copy(F[:, 1:2], u[:])
            nc.vector.tensor_mul(F[:, 2:3], F[:, 1:2], F[:, 0:1])
            nc.vector.tensor_mul(F[:, 3:4], F[:, 1:2], F[:, 1:2])
            nc.vector.tensor_mul(F[:, 4:5], F[:, 3:4], F[:, 0:1])
            nc.vector.tensor_mul(F[:, 5:6], F[:, 3:4], F[:, 1:2])

            for k in range(6):
                nc.vector.tensor_scalar_mul(
                    W[g][:, BPG * k:BPG * (k + 1)], wsel_t[:], F[:, k:k + 1])

        # ---- phase 2: matmul (scale+mean fused in weights) + DMA out ----
        stage_pool = ctx.enter_context(tc.tile_pool(name="stage", bufs=3))
        cp_flip = 0
        for g in range(N_GROUPS):
            for (p0, p1, members) in mm_tiles:
                ps = psum_pool.tile([BPG, cfg["psum_cols"]], F32, name="ps", tag="ps")
                for (k, a, b) in members:
                    lhsT = W[g][:, BPG * k:BPG * (k + 1)]
                    rhs = XG[g][:, a:b]
                    if mm_dt != F32:
                        lhsT = lhsT.bitcast(mm_dt)
                        rhs = rhs.bitcast(mm_dt)
                    nc.tensor.matmul(
                        ps[:, a - p0:b - p0], lhsT, rhs,
                        start=True, stop=True)
                st = stage_pool.tile([BPG, cfg["psum_cols"]], F32, name="st", tag="st")
                if cp_flip % 2 == 0:
                    nc.vector.tensor_copy(st[:, : p1 - p0], ps[:, : p1 - p0])
                else:
                    nc.scalar.copy(st[:, : p1 - p0], ps[:, : p1 - p0])
                cp_flip += 1
                nc.sync.dma_start(
                    out[g * BPG:(g + 1) * BPG, p0:p1], st[:, : p1 - p0])

    nc.compile()
    return nc


def _get_nc(cfg_key=None):
    key = cfg_key or tuple(sorted(CONFIG.items()))
    if key not in _cache:
        _cache[key] = _build_kernel(CONFIG)
    return _cache[key]


def _wsel_np():
    w = np.zeros((128, BPG), dtype=np.float32)
    for j in range(BPG):
        w[j * 32:(j + 1) * 32, j] = 1.0 / 32.0
    return w


def kernel(signatures: np.ndarray, **_ignored) -> np.ndarray:
    x = np.ascontiguousarray(np.asarray(signatures), dtype=np.float32)
    assert x.shape == (B, S, L), x.shape
    nc = _get_nc()
    wsel = _wsel_np()
    in_maps = [
        {"x": np.ascontiguousarray(x[i * B_LOC:(i + 1) * B_LOC].reshape(ROWS, L)),
         "wsel": wsel}
        for i in range(N_CORES)
    ]
    res = bass_utils.run_bass_kernel_spmd(nc, in_maps, core_ids=list(range(N_CORES)))
    out = np.concatenate([res.results[i]["out"] for i in range(N_CORES)], axis=0)
    return out.astype(np.float32, copy=False)


if __name__ == "__main__":
    rng = np.random.default_rng(0)
    sig = rng.standard_normal((B, S, L), dtype=np.float32) * 0.5
    o = kernel(signatures=sig)
    print("out", o.shape, o.dtype, float(np.abs(o).max()))


# revision 9
# speedup vs baseline: 1.2874x; 1.2874x over previous
"""Trainium2 Bass kernel for nn_ExpectedSignature.

Computes, for signatures x[B=64, S=32, L=19530] (L = sum_{k=1..6} 5^k):
  1. per-(b,s) level sums  l_k = sum_{i in level k} x_i^2
  2. c0 = 1 - phi(1 + sum_k l_k)   (phi(x) = x for x<=4 else 8 - 16/x)
  3. root u of  c0 + sum_k l_k u^k = 0  on [0,1]  (u = t^2, t = dilatation norm)
  4. out[b, i] = mean_s x[b,s,i] * t^{level(i)}

Sharding: data-parallel over batch, 8 batches per core on 8 cores.

Per-core pipeline (rows (b_local*32+s) -> 2 partition groups of 128 rows):
  - all input DMAs issue first (HWDGE stays saturated)
  - phase 1 (per group): fused square+accumulate chunks split across the
    Vector (scalar_tensor_tensor) and Scalar (activation Square) engines
  - solve (per group, Vector-only): Newton on u with an exponent-bit-trick
    6th-root seed, fused p/q Horner via scalar_tensor_tensor on [128,2]
    tiles, bit-trick sqrt + 2 Newton refines; no activation tables needed
  - phase 2 (per group): column-tiled fp32 matmuls -- 4 concurrent 32-row
    strips of the PE array write one PSUM bank [128,512] covering 2048
    output columns; stationary weights (batch-onehot/32)*t^level fuse the
    scaling and the sample mean; [128,512] staging copies then DMA out.
"""

import math
from contextlib import ExitStack

import numpy as np

import concourse.bass as bass
import concourse.bacc as bacc
import concourse.mybir as mybir
import concourse.tile as tile
from concourse import bass_utils

F32 = mybir.dt.float32
F32R = mybir.dt.float32r
I32 = mybir.dt.int32
AF = mybir.ActivationFunctionType
ALU = mybir.AluOpType
AX = mybir.AxisListType

B, S, L = 64, 32, 19530
N_CORES = 8
B_LOC = B // N_CORES          # 8 batches per core
ROWS = B_LOC * S              # 256 rows per core
N_GROUPS = 2                  # 2 partition groups of 128 rows
BPG = 4                       # batches per group
LEVEL_STARTS = [0, 5, 30, 155, 780, 3905, 19530]

MU = 0.0450465
K6 = float((1.0 - 1.0 / 6.0) * (127.0 - MU) * (1 << 23))
K2 = float(0.5 * (127.0 - MU) * (1 << 23))

CONFIG = {
    "n_newton": 4,
    "chunk": 1302,            # phase-1 compute/DMA chunk (columns)
    "mm_mode": "coltile",     # "coltile" | "plain"
    "psum_cols": 512,         # PSUM tile free size (one bank)
    "psum_bufs": 4,
    "stage_bufs": 4,
}

_cache = {}


def _chunk_plan(chunk):
    """Per level, split [start, end) into pieces <= chunk: (level, c0, c1)."""
    plan = []
    for k in range(6):
        c0, c1 = LEVEL_STARTS[k], LEVEL_STARTS[k + 1]
        n = c1 - c0
        pieces = max(1, math.ceil(n / chunk))
        base, rem = divmod(n, pieces)
        a = c0
        for p in range(pieces):
            sz = base + (1 if p < rem else 0)
            plan.append((k, a, a + sz))
            a += sz
        assert a == c1
    return plan


def _assign_engines(plan):
    """'v' (vector) or 's' (scalar) per chunk, balanced ~half/half."""
    eng = []
    flip = 0
    for (k, a, b) in plan:
        if k <= 2:
            eng.append("v")       # tiny levels: cheap on vector
        elif k == 3:
            eng.append("s")
        else:
            eng.append("s" if flip % 2 == 0 else "v")
            flip += 1
    return eng


def _segments():
    """Column segments split at level boundaries + the 512 grid: (k, a, b)."""
    bounds = sorted(set(LEVEL_STARTS) | set(range(0, L + 1, 512)) | {L})
    segs = []
    for a, b in zip(bounds[:-1], bounds[1:]):
        k = next(i for i in range(6) if LEVEL_STARTS[i] <= a < LEVEL_STARTS[i + 1])
        segs.append((k, a, b))
    return segs


def _build_kernel(cfg):
    nc = bacc.Bacc(
        "TRN2", target_bir_lowering=False, debug=False, num_devices=N_CORES)
    x = nc.dram_tensor("x", [ROWS, L], F32, kind="ExternalInput").ap()
    wselr = nc.dram_tensor("wselr", [128, 192], F32, kind="ExternalInput").ap()
    out = nc.dram_tensor("out", [B_LOC, L], F32, kind="ExternalOutput").ap()

    plan = _chunk_plan(cfg["chunk"])
    engines = _assign_engines(plan)
    n_chunks = len(plan)
    segs = _segments()
    lvl_rng = []
    for k in range(6):
        idxs = [i for i, (kk, _, _) in enumerate(plan) if kk == k]
        lvl_rng.append((min(idxs), max(idxs) + 1))

    with ExitStack() as ctx:
        tc = ctx.enter_context(tile.TileContext(nc))
        xg_pool = ctx.enter_context(tc.tile_pool(name="xg", bufs=1))
        cst = ctx.enter_context(tc.tile_pool(name="cst", bufs=1))
        scr_v = ctx.enter_context(tc.tile_pool(name="scr_v", bufs=2))
        scr_s = ctx.enter_context(tc.tile_pool(name="scr_s", bufs=2))
        sol = ctx.enter_context(tc.tile_pool(name="sol", bufs=1))
        psum_pool = ctx.enter_context(
            tc.tile_pool(name="psum", bufs=cfg["psum_bufs"], space="PSUM"))
        stage = ctx.enter_context(tc.tile_pool(name="stage", bufs=cfg["stage_bufs"]))

        wsel_t = cst.tile([128, 192], F32, name="wsel_t")
        nc.sync.dma_start(wsel_t[:], wselr)
        kmul = cst.tile([128, 6], F32, name="kmul")
        for j in range(6):
            nc.vector.memset(kmul[:, j:j + 1], float(j + 1))

        XG, PART, LVW, W = [], [], [], []
        for g in range(N_GROUPS):
            XG.append(xg_pool.tile([128, L], F32, name=f"xg{g}"))
            PART.append(cst.tile([128, n_chunks], F32, name=f"part{g}"))
            # LVW cols: 0..5 l_k | 6 c0 | 7..12 k*l_k | 13 zero
            LVW.append(cst.tile([128, 14], F32, name=f"lvw{g}"))
            W.append(cst.tile([128, 192], F32, name=f"w{g}"))

        # ---- all input DMAs first ----
        for g in range(N_GROUPS):
            rows = slice(g * 128, (g + 1) * 128)
            for (k, a, b) in plan:
                nc.sync.dma_start(XG[g][:, a:b], x[rows, a:b])

        cp_state = [0]

        def emit_phase1(g):
            for ci, (k, a, b) in enumerate(plan):
                xt = XG[g][:, a:b]
                acc = PART[g][:, ci:ci + 1]
                if engines[ci] == "v":
                    scr = scr_v.tile([128, cfg["chunk"]], F32, name="scrv",
                                     tag="scr_v")
                    nc.vector.scalar_tensor_tensor(
                        out=scr[:, : b - a], in0=xt, scalar=1.0, in1=xt,
                        op0=ALU.bypass, op1=ALU.mult, accum_out=acc)
                else:
                    scr = scr_s.tile([128, cfg["chunk"]], F32, name="scrs",
                                     tag="scr_s")
                    nc.scalar.activation(
                        out=scr[:, : b - a], in_=xt, func=AF.Square,
                        accum_out=acc)

        def emit_solve(g):
            lvw = LVW[g]
            for k in range(6):
                lo, hi = lvl_rng[k]
                if hi - lo == 1:
                    nc.vector.tensor_copy(lvw[:, k:k + 1], PART[g][:, lo:hi])
                else:
                    nc.vector.tensor_reduce(
                        out=lvw[:, k:k + 1], in_=PART[g][:, lo:hi],
                        axis=AX.X, op=ALU.add)
            sl = sol.tile([128, 12], F32, name=f"sl{g}")
            ua = sol.tile([128, 1], F32, name=f"ua{g}")
            ub = sol.tile([128, 1], F32, name=f"ub{g}")
            pq = sol.tile([128, 2], F32, name=f"pq{g}")
            Ft = sol.tile([128, 6], F32, name=f"ft{g}")

            sumlv, nq, rnq, c0b = sl[:, 0:1], sl[:, 1:2], sl[:, 2:3], sl[:, 3:4]
            c0s, msk, dlt, rl6 = sl[:, 4:5], sl[:, 5:6], sl[:, 6:7], sl[:, 7:8]
            t1, bf, yy, tnew = sl[:, 8:9], sl[:, 9:10], sl[:, 10:11], sl[:, 11:12]

            nc.vector.tensor_reduce(out=sumlv, in_=lvw[:, 0:6], axis=AX.X,
                                    op=ALU.add)
            nc.vector.tensor_scalar(nq, sumlv, 1.0, None, ALU.add)
            nc.vector.reciprocal(rnq, nq)
            nc.vector.tensor_scalar(c0b, rnq, 16.0, -7.0, ALU.mult, ALU.add)
            nc.vector.tensor_scalar(c0s, nq, -1.0, 1.0, ALU.mult, ALU.add)
            nc.vector.tensor_scalar(msk, nq, 4.0, None, ALU.is_gt)
            nc.vector.tensor_sub(dlt, c0b, c0s)
            nc.vector.scalar_tensor_tensor(
                lvw[:, 6:7], dlt, msk[:, 0:1], c0s, op0=ALU.mult, op1=ALU.add)
            nc.vector.memset(lvw[:, 13:14], 0.0)
            nc.vector.tensor_tensor(lvw[:, 7:13], lvw[:, 0:6], kmul[:], ALU.mult)

            # seed u0 = min(1, (-c0/l6)^(1/6)) via exponent bit trick
            nc.vector.reciprocal(rl6, lvw[:, 5:6])
            nc.vector.scalar_tensor_tensor(
                t1, lvw[:, 6:7], -1.0, rl6, op0=ALU.mult, op1=ALU.mult)
            nc.vector.tensor_copy(bf, t1.bitcast(I32))       # int->float value
            nc.vector.tensor_scalar(yy, bf, 1.0 / 6.0, K6, ALU.mult, ALU.add)
            nc.vector.tensor_copy(t1.bitcast(I32), yy)       # float->int value
            nc.vector.tensor_scalar_min(ua, t1, 1.0)

            def pair(k):
                c = k - 1 if k >= 1 else 6
                return lvw[:, c:c + 8:7]

            u, un = ua, ub
            for it in range(cfg["n_newton"]):
                nc.vector.scalar_tensor_tensor(
                    pq[:], pair(6), u[:, 0:1], pair(5), op0=ALU.mult,
                    op1=ALU.add)
                for k in (4, 3, 2, 1, 0):
                    nc.vector.scalar_tensor_tensor(
                        pq[:], pq[:], u[:, 0:1], pair(k), op0=ALU.mult,
                        op1=ALU.add)
                nc.vector.tensor_sub(dlt, pq[:, 1:2], pq[:, 0:1])
                nc.vector.reciprocal(rnq, pq[:, 1:2])
                nc.vector.scalar_tensor_tensor(
                    un[:], dlt, rnq[:, 0:1], u[:], op0=ALU.mult, op1=ALU.mult)
                u, un = un, u

            # t = min(1, sqrt(u)): bit-trick seed + 2 Newton refines
            nc.vector.tensor_copy(bf, u.bitcast(I32))
            nc.vector.tensor_scalar(yy, bf, 0.5, K2, ALU.mult, ALU.add)
            nc.vector.tensor_copy(t1.bitcast(I32), yy)
            tcur = t1
            for r in range(2):
                nxt = tnew if tcur is t1 else t1
                nc.vector.reciprocal(rnq, tcur)
                nc.vector.scalar_tensor_tensor(
                    dlt, rnq, u[:, 0:1], tcur, op0=ALU.mult, op1=ALU.add)
                nc.vector.tensor_scalar(nxt, dlt, 0.5, None, ALU.mult)
                tcur = nxt
            # F = (t, u, ut, u2, u2t, u3)
            nc.vector.tensor_scalar_min(Ft[:, 0:1], tcur, 1.0)
            nc.vector.tensor_copy(Ft[:, 1:2], u[:])
            nc.vector.tensor_scalar(Ft[:, 2:4], Ft[:, 0:2], u[:, 0:1], None,
                                    ALU.mult)
            nc.vector.tensor_scalar(Ft[:, 4:6], Ft[:, 2:4], u[:, 0:1], None,
                                    ALU.mult)
            # W[:, 32k+m] = wsel[:, 32k+m] * F[:, k]  (cols m>=4 are zero)
            fb = Ft[:].unsqueeze(2).broadcast_to([128, 6, 32])
            nc.vector.tensor_tensor(W[g][:], wsel_t[:], fb, ALU.mult)

        def emit_phase2(g):
            pc = cfg["psum_cols"]
            span = 4 * pc    # out-columns covered per PSUM tile
            if cfg["mm_mode"] == "plain":
                for p0 in range(0, L, 1024):
                    p1 = min(p0 + 1024, L)
                    ps = psum_pool.tile([BPG, 1024], F32, name="ps", tag="ps")
                    for (k, a, b) in segs:
                        if a < p0 or b > p1:
                            continue
                        nc.tensor.matmul(
                            ps[:, a - p0:b - p0],
                            W[g][:, 32 * k:32 * k + BPG],
                            XG[g][:, a:b], start=True, stop=True)
                    st = stage.tile([BPG, 1024], F32, name="st", tag="st")
                    if cp_state[0] % 2 == 0:
                        nc.vector.tensor_copy(st[:, : p1 - p0], ps[:, : p1 - p0])
                    else:
                        nc.scalar.copy(st[:, : p1 - p0], ps[:, : p1 - p0])
                    cp_state[0] += 1
                    nc.sync.dma_start(
                        out[g * BPG:(g + 1) * BPG, p0:p1], st[:, : p1 - p0])
                return
            for tile0 in range(0, L, span):
                tile1 = min(tile0 + span, L)
                ps = psum_pool.tile([128, pc], F32, name="ps", tag="ps")
                strips = []
                for j in range(4):
                    s0 = tile0 + j * pc
                    s1 = min(s0 + pc, tile1)
                    if s0 >= s1:
                        break
                    strips.append((j, s0, s1))
                    for (k, a, b) in segs:
                        if a < s0 or b > s1:
                            continue
                        nc.tensor.matmul(
                            ps[32 * j:32 * j + 32, a - s0:b - s0],
                            W[g][:, 32 * k:32 * (k + 1)], XG[g][:, a:b],
                            start=True, stop=True,
                            tile_position=(0, 32 * j))
                st = stage.tile([128, pc], F32, name="st", tag="st")
                full = all(s1 - s0 == pc for (_, s0, s1) in strips) \
                    and len(strips) == 4
                if full:
                    if cp_state[0] % 2 == 0:
                        nc.vector.tensor_copy(st[:, :], ps[:, :])
                    else:
                        nc.scalar.copy(st[:, :], ps[:, :])
                    cp_state[0] += 1
                else:
                    for (j, s0, s1) in strips:
                        w_ = s1 - s0
                        if cp_state[0] % 2 == 0:
                            nc.vector.tensor_copy(
                                st[32 * j:32 * j + BPG, :w_],
                                ps[32 * j:32 * j + BPG, :w_])
                        else:
                            nc.scalar.copy(
                                st[32 * j:32 * j + BPG, :w_],
                                ps[32 * j:32 * j + BPG, :w_])
                        cp_state[0] += 1
                for (j, s0, s1) in strips:
                    nc.sync.dma_start(
                        out[g * BPG:(g + 1) * BPG, s0:s1],
                        st[32 * j:32 * j + BPG, : s1 - s0])

        for g in range(N_GROUPS):
            emit_phase1(g)
            emit_solve(g)
            emit_phase2(g)

    nc.compile()
    return nc


def _get_nc():
    key = tuple(sorted((k, str(v)) for k, v in CONFIG.items()))
    if key not in _cache:
        _cache[key] = _build_kernel(CONFIG)
    return _cache[key]


def _wsel_np():
    w = np.zeros((128, 192), dtype=np.float32)
    for k in range(6):
        for j in range(BPG):
            w[j * 32:(j + 1) * 32, 32 * k + j] = 1.0 / 32.0
    return w


def kernel(signatures: np.ndarray, **_ignored) -> np.ndarray:
    x = np.ascontiguousarray(np.asarray(signatures), dtype=np.float32)
    assert x.shape == (B, S, L), x.shape
    nc = _get_nc()
    wsel = _wsel_np()
    in_maps = [
        {"x": np.ascontiguousarray(x[i * B_LOC:(i + 1) * B_LOC].reshape(ROWS, L)),
         "wselr": wsel}
        for i in range(N_CORES)
    ]
    res = bass_utils.run_bass_kernel_spmd(nc, in_maps, core_ids=list(range(N_CORES)))
    out = np.concatenate([res.results[i]["out"] for i in range(N_CORES)], axis=0)
    return out.astype(np.float32, copy=False)


if __name__ == "__main__":
    rng = np.random.default_rng(0)
    sig = rng.standard_normal((B, S, L), dtype=np.float32) * 0.5
    o = kernel(signatures=sig)
    print("out", o.shape, o.dtype, float(np.abs(o).max()))


# revision 12
# speedup vs baseline: 1.3444x; 1.0443x over previous
"""Trainium2 Bass kernel for nn_ExpectedSignature.

Computes, for signatures x[B=64, S=32, L=19530] (L = sum_{k=1..6} 5^k):
  1. per-(b,s) level sums  l_k = sum_{i in level k} x_i^2
  2. c0 = 1 - phi(1 + sum_k l_k)   (phi(x) = x for x<=4 else 8 - 16/x)
  3. root u of  c0 + sum_k l_k u^k = 0  on [0,1]  (u = t^2, t = dilatation norm)
  4. out[b, i] = mean_s x[b,s,i] * t^{level(i)}

Sharding: data-parallel over batch, 8 batches per core on 8 cores.

Per-core pipeline (rows (b_local*32+s) -> 2 partition groups of 128 rows):
  - all input DMAs issue first (HWDGE stays saturated)
  - phase 1 (per group): fused square+accumulate chunks split across the
    Vector (scalar_tensor_tensor) and Scalar (activation Square) engines
  - solve (per group, Vector-only): Newton on u with an exponent-bit-trick
    6th-root seed, fused p/q Horner via scalar_tensor_tensor on [128,2]
    tiles, bit-trick sqrt + 2 Newton refines; no activation tables needed
  - phase 2 (per group): column-tiled fp32 matmuls -- 4 concurrent 32-row
    strips of the PE array write one PSUM bank [128,512] covering 2048
    output columns; stationary weights (batch-onehot/32)*t^level fuse the
    scaling and the sample mean; [128,512] staging copies then DMA out.
"""

import math
from contextlib import ExitStack

import numpy as np

import concourse.bass as bass
import concourse.bacc as bacc
import concourse.mybir as mybir
import concourse.tile as tile
from concourse import bass_utils

F32 = mybir.dt.float32
F32R = mybir.dt.float32r
I32 = mybir.dt.int32
AF = mybir.ActivationFunctionType
ALU = mybir.AluOpType
AX = mybir.AxisListType

B, S, L = 64, 32, 19530
N_CORES = 8
B_LOC = B // N_CORES          # 8 batches per core
ROWS = B_LOC * S              # 256 rows per core
N_GROUPS = 2                  # 2 partition groups of 128 rows
BPG = 4                       # batches per group
LEVEL_STARTS = [0, 5, 30, 155, 780, 3905, 19530]

MU = 0.0450465
K6 = float((1.0 - 1.0 / 6.0) * (127.0 - MU) * (1 << 23))
K2 = float(0.5 * (127.0 - MU) * (1 << 23))

CONFIG = {
    "n_newton": 4,
    "chunk": 1024,            # phase-1 compute chunk (columns)
    "dma_cols": 2700,         # target input-DMA piece size (merged chunks)
    "mm_mode": "coltile",     # "coltile" | "plain"
    "psum_cols": 512,         # PSUM tile free size (one bank)
    "psum_bufs": 4,
    "stage_bufs": 2,
    "stage_span": 4,          # psum tiles per staging tile
}

_cache = {}


def _chunk_plan(chunk):
    """Per level, split [start, end) into pieces <= chunk: (level, c0, c1)."""
    plan = []
    for k in range(6):
        c0, c1 = LEVEL_STARTS[k], LEVEL_STARTS[k + 1]
        n = c1 - c0
        pieces = max(1, math.ceil(n / chunk))
        base, rem = divmod(n, pieces)
        a = c0
        for p in range(pieces):
            sz = base + (1 if p < rem else 0)
            plan.append((k, a, a + sz))
            a += sz
        assert a == c1
    return plan


def _assign_engines(plan):
    """'v' (vector) or 's' (scalar) per chunk, balanced ~half/half."""
    eng = []
    flip = 0
    for (k, a, b) in plan:
        if k <= 2:
            eng.append("v")       # tiny levels: cheap on vector
        elif k == 3:
            eng.append("s")
        else:
            eng.append("s" if flip % 2 == 0 else "v")
            flip += 1
    return eng


def _dma_plan(plan, target):
    """Merge consecutive compute chunks into DMA pieces ~target columns."""
    pieces = []
    cur0, cur1 = None, None
    for (_, a, b) in plan:
        if cur0 is None:
            cur0, cur1 = a, b
        elif cur1 - cur0 >= target:
            pieces.append((cur0, cur1))
            cur0, cur1 = a, b
        else:
            cur1 = b
    pieces.append((cur0, cur1))
    return pieces


def _segments():
    """Column segments split at level boundaries + the 512 grid: (k, a, b)."""
    bounds = sorted(set(LEVEL_STARTS) | set(range(0, L + 1, 512)) | {L})
    segs = []
    for a, b in zip(bounds[:-1], bounds[1:]):
        k = next(i for i in range(6) if LEVEL_STARTS[i] <= a < LEVEL_STARTS[i + 1])
        segs.append((k, a, b))
    return segs


def _build_kernel(cfg):
    nc = bacc.Bacc(
        "TRN2", target_bir_lowering=False, debug=False, num_devices=N_CORES)
    x = nc.dram_tensor("x", [ROWS, L], F32, kind="ExternalInput").ap()
    wselr = nc.dram_tensor("wselr", [128, 192], F32, kind="ExternalInput").ap()
    out = nc.dram_tensor("out", [B_LOC, L], F32, kind="ExternalOutput").ap()

    plan = _chunk_plan(cfg["chunk"])
    engines = _assign_engines(plan)
    segs = _segments()
    dma_pieces = _dma_plan(plan, cfg["dma_cols"])
    # PART layout: level k chunk j -> column NCHK*k + j (zero-padded)
    NCHK = max(sum(1 for (kk, _, _) in plan if kk == k) for k in range(6))
    part_col = {}
    ctr = [0] * 6
    for ci, (k, a, b) in enumerate(plan):
        part_col[ci] = NCHK * k + ctr[k]
        ctr[k] += 1

    with ExitStack() as ctx:
        tc = ctx.enter_context(tile.TileContext(nc))
        xg_pool = ctx.enter_context(tc.tile_pool(name="xg", bufs=1))
        cst = ctx.enter_context(tc.tile_pool(name="cst", bufs=1))
        scr_v = ctx.enter_context(tc.tile_pool(name="scr_v", bufs=2))
        scr_s = ctx.enter_context(tc.tile_pool(name="scr_s", bufs=2))
        sol = ctx.enter_context(tc.tile_pool(name="sol", bufs=1))
        psum_pool = ctx.enter_context(
            tc.tile_pool(name="psum", bufs=cfg["psum_bufs"], space="PSUM"))
        stage = ctx.enter_context(tc.tile_pool(name="stage", bufs=cfg["stage_bufs"]))

        wsel_t = cst.tile([128, 192], F32, name="wsel_t")
        nc.sync.dma_start(wsel_t[:], wselr)
        kmul = cst.tile([128, 6], F32, name="kmul")
        for j in range(6):
            nc.vector.memset(kmul[:, j:j + 1], float(j + 1))

        XG, PART, LVW, W = [], [], [], []
        for g in range(N_GROUPS):
            XG.append(xg_pool.tile([128, L], F32, name=f"xg{g}"))
            PART.append(cst.tile([128, 6 * NCHK], F32, name=f"part{g}"))
            # LVW cols: 0..5 l_k | 6 c0 | 7..12 k*l_k | 13 zero
            LVW.append(cst.tile([128, 14], F32, name=f"lvw{g}"))
            W.append(cst.tile([128, 192], F32, name=f"w{g}"))

        for g in range(N_GROUPS):
            nc.vector.memset(PART[g][:], 0.0)

        # ---- all input DMAs first (big merged pieces) ----
        for g in range(N_GROUPS):
            rows = slice(g * 128, (g + 1) * 128)
            for (a, b) in dma_pieces:
                nc.sync.dma_start(XG[g][:, a:b], x[rows, a:b])

        cp_state = [0]

        def emit_phase1(g):
            for ci, (k, a, b) in enumerate(plan):
                xt = XG[g][:, a:b]
                pc_ = part_col[ci]
                acc = PART[g][:, pc_:pc_ + 1]
                if engines[ci] == "v":
                    scr = scr_v.tile([128, cfg["chunk"]], F32, name="scrv",
                                     tag="scr_v")
                    nc.vector.scalar_tensor_tensor(
                        out=scr[:, : b - a], in0=xt, scalar=1.0, in1=xt,
                        op0=ALU.bypass, op1=ALU.mult, accum_out=acc)
                else:
                    scr = scr_s.tile([128, cfg["chunk"]], F32, name="scrs",
                                     tag="scr_s")
                    nc.scalar.activation(
                        out=scr[:, : b - a], in_=xt, func=AF.Square,
                        accum_out=acc)

        def emit_solve(g):
            lvw = LVW[g]
            nc.vector.tensor_reduce(
                out=lvw[:, 0:6],
                in_=PART[g][:].rearrange("p (k j) -> p k j", j=NCHK),
                axis=AX.X, op=ALU.add)
            sl = sol.tile([128, 12], F32, name=f"sl{g}")
            ua = sol.tile([128, 1], F32, name=f"ua{g}")
            ub = sol.tile([128, 1], F32, name=f"ub{g}")
            pq = sol.tile([128, 2], F32, name=f"pq{g}")
            Ft = sol.tile([128, 6], F32, name=f"ft{g}")

            sumlv, nq, rnq, c0b = sl[:, 0:1], sl[:, 1:2], sl[:, 2:3], sl[:, 3:4]
            c0s, msk, dlt, rl6 = sl[:, 4:5], sl[:, 5:6], sl[:, 6:7], sl[:, 7:8]
            t1, bf, yy, tnew = sl[:, 8:9], sl[:, 9:10], sl[:, 10:11], sl[:, 11:12]

            nc.vector.tensor_reduce(out=sumlv, in_=lvw[:, 0:6], axis=AX.X,
                                    op=ALU.add)
            nc.vector.tensor_scalar(nq, sumlv, 1.0, None, ALU.add)
            nc.vector.reciprocal(rnq, nq)
            nc.vector.tensor_scalar(c0b, rnq, 16.0, -7.0, ALU.mult, ALU.add)
            nc.vector.tensor_scalar(c0s, nq, -1.0, 1.0, ALU.mult, ALU.add)
            nc.vector.tensor_scalar(msk, nq, 4.0, None, ALU.is_gt)
            nc.vector.tensor_sub(dlt, c0b, c0s)
            nc.vector.scalar_tensor_tensor(
                lvw[:, 6:7], dlt, msk[:, 0:1], c0s, op0=ALU.mult, op1=ALU.add)
            nc.vector.memset(lvw[:, 13:14], 0.0)
            nc.vector.tensor_tensor(lvw[:, 7:13], lvw[:, 0:6], kmul[:], ALU.mult)

            # seed u0 = min(1, (-c0/l6)^(1/6)) via exponent bit trick
            nc.vector.reciprocal(rl6, lvw[:, 5:6])
            nc.vector.scalar_tensor_tensor(
                t1, lvw[:, 6:7], -1.0, rl6, op0=ALU.mult, op1=ALU.mult)
            nc.vector.tensor_copy(bf, t1.bitcast(I32))       # int->float value
            nc.vector.tensor_scalar(yy, bf, 1.0 / 6.0, K6, ALU.mult, ALU.add)
            nc.vector.tensor_copy(t1.bitcast(I32), yy)       # float->int value
            nc.vector.tensor_scalar_min(ua, t1, 1.0)

            def pair(k):
                c = k - 1 if k >= 1 else 6
                return lvw[:, c:c + 8:7]

            u, un = ua, ub
            for it in range(cfg["n_newton"]):
                nc.vector.scalar_tensor_tensor(
                    pq[:], pair(6), u[:, 0:1], pair(5), op0=ALU.mult,
                    op1=ALU.add)
                for k in (4, 3, 2, 1, 0):
                    nc.vector.scalar_tensor_tensor(
                        pq[:], pq[:], u[:, 0:1], pair(k), op0=ALU.mult,
                        op1=ALU.add)
                nc.vector.tensor_sub(dlt, pq[:, 1:2], pq[:, 0:1])
                nc.vector.reciprocal(rnq, pq[:, 1:2])
                nc.vector.scalar_tensor_tensor(
                    un[:], dlt, rnq[:, 0:1], u[:], op0=ALU.mult, op1=ALU.mult)
                u, un = un, u

            # t = min(1, sqrt(u)): bit-trick seed + 2 Newton refines
            nc.vector.tensor_copy(bf, u.bitcast(I32))
            nc.vector.tensor_scalar(yy, bf, 0.5, K2, ALU.mult, ALU.add)
            nc.vector.tensor_copy(t1.bitcast(I32), yy)
            tcur = t1
            for r in range(2):
                nxt = tnew if tcur is t1 else t1
                nc.vector.reciprocal(rnq, tcur)
                nc.vector.scalar_tensor_tensor(
                    dlt, rnq, u[:, 0:1], tcur, op0=ALU.mult, op1=ALU.add)
                nc.vector.tensor_scalar(nxt, dlt, 0.5, None, ALU.mult)
                tcur = nxt
            # F = (t, u, ut, u2, u2t, u3)
            nc.vector.tensor_scalar_min(Ft[:, 0:1], tcur, 1.0)
            nc.vector.tensor_copy(Ft[:, 1:2], u[:])
            nc.vector.tensor_scalar(Ft[:, 2:4], Ft[:, 0:2], u[:, 0:1], None,
                                    ALU.mult)
            nc.vector.tensor_scalar(Ft[:, 4:6], Ft[:, 2:4], u[:, 0:1], None,
                                    ALU.mult)
            # W[:, 32k+m] = wsel[:, 32k+m] * F[:, k]  (cols m>=4 are zero)
            fb = Ft[:].unsqueeze(2).broadcast_to([128, 6, 32])
            nc.vector.tensor_tensor(W[g][:], wsel_t[:], fb, ALU.mult)

        def emit_phase2(g):
            pc = cfg["psum_cols"]
            span = 4 * pc    # out-columns covered per PSUM tile
            if cfg["mm_mode"] == "plain":
                for p0 in range(0, L, 1024):
                    p1 = min(p0 + 1024, L)
                    ps = psum_pool.tile([BPG, 1024], F32, name="ps", tag="ps")
                    for (k, a, b) in segs:
                        if a < p0 or b > p1:
                            continue
                        nc.tensor.matmul(
                            ps[:, a - p0:b - p0],
                            W[g][:, 32 * k:32 * k + BPG],
                            XG[g][:, a:b], start=True, stop=True)
                    st = stage.tile([BPG, 1024], F32, name="st", tag="st")
                    if cp_state[0] % 2 == 0:
                        nc.vector.tensor_copy(st[:, : p1 - p0], ps[:, : p1 - p0])
                    else:
                        nc.scalar.copy(st[:, : p1 - p0], ps[:, : p1 - p0])
                    cp_state[0] += 1
                    nc.sync.dma_start(
                        out[g * BPG:(g + 1) * BPG, p0:p1], st[:, : p1 - p0])
                return
            nspan = cfg["stage_span"]
            big = nspan * span      # out-columns covered per staging tile
            for big0 in range(0, L, big):
                big1 = min(big0 + big, L)
                st = stage.tile([128, nspan * pc], F32, name="st", tag="st")
                mtiles = []
                for m, tile0 in enumerate(range(big0, big1, span)):
                    tile1 = min(tile0 + span, big1)
                    ps = psum_pool.tile([128, pc], F32, name="ps", tag="ps")
                    strips = []
                    for j in range(4):
                        s0 = tile0 + j * pc
                        s1 = min(s0 + pc, tile1)
                        if s0 >= s1:
                            break
                        strips.append((j, s0, s1))
                        for (k, a, b) in segs:
                            if a < s0 or b > s1:
                                continue
                            nc.tensor.matmul(
                                ps[32 * j:32 * j + 32, a - s0:b - s0],
                                W[g][:, 32 * k:32 * (k + 1)], XG[g][:, a:b],
                                start=True, stop=True,
                                tile_position=(0, 32 * j))
                    full = len(strips) == 4 and all(
                        s1 - s0 == pc for (_, s0, s1) in strips)
                    if full:
                        if cp_state[0] % 2 == 0:
                            nc.vector.tensor_copy(
                                st[:, m * pc:(m + 1) * pc], ps[:, :])
                        else:
                            nc.scalar.copy(
                                st[:, m * pc:(m + 1) * pc], ps[:, :])
                        cp_state[0] += 1
                    else:
                        for (j, s0, s1) in strips:
                            w_ = s1 - s0
                            if cp_state[0] % 2 == 0:
                                nc.vector.tensor_copy(
                                    st[32 * j:32 * j + BPG,
                                       m * pc:m * pc + w_],
                                    ps[32 * j:32 * j + BPG, :w_])
                            else:
                                nc.scalar.copy(
                                    st[32 * j:32 * j + BPG,
                                       m * pc:m * pc + w_],
                                    ps[32 * j:32 * j + BPG, :w_])
                            cp_state[0] += 1
                    mtiles.append((m, tile0, tile1, strips))
                # batched out-DMAs: one per strip j across the m sub-tiles
                nm = len(mtiles)
                uniform = all(
                    len(strips) == 4 and all(s1 - s0 == pc for (_, s0, s1) in strips)
                    for (_, _, _, strips) in mtiles)
                if uniform and nm > 1:
                    for j in range(4):
                        src_ap = st[32 * j:32 * j + BPG, 0:nm * pc] \
                            .rearrange("p (m c) -> p m c", c=pc)
                        # dst columns for strip j: big0 + m*span + j*pc + c
                        dst_ap = bass.AP(
                            out.tensor,
                            (g * BPG) * L + big0 + j * pc,
                            [[L, BPG], [span, nm], [1, pc]])
                        nc.sync.dma_start(dst_ap, src_ap)
                else:
                    for (m, tile0, tile1, strips) in mtiles:
                        for (j, s0, s1) in strips:
                            nc.sync.dma_start(
                                out[g * BPG:(g + 1) * BPG, s0:s1],
                                st[32 * j:32 * j + BPG,
                                   m * pc:m * pc + (s1 - s0)])

        for g in range(N_GROUPS):
            emit_phase1(g)
            emit_solve(g)
            emit_phase2(g)

    nc.compile()
    return nc


def _get_nc():
    key = tuple(sorted((k, str(v)) for k, v in CONFIG.items()))
    if key not in _cache:
        _cache[key] = _build_kernel(CONFIG)
    return _cache[key]


def _wsel_np():
    w = np.zeros((128, 192), dtype=np.float32)
    for k in range(6):
        for j in range(BPG):
            w[j * 32:(j + 1) * 32, 32 * k + j] = 1.0 / 32.0
    return w


def kernel(signatures: np.ndarray, **_ignored) -> np.ndarray:
    x = np.ascontiguousarray(np.asarray(signatures), dtype=np.float32)
    assert x.shape == (B, S, L), x.shape
    nc = _get_nc()
    wsel = _wsel_np()
    in_maps = [
        {"x": np.ascontiguousarray(x[i * B_LOC:(i + 1) * B_LOC].reshape(ROWS, L)),
         "wselr": wsel}
        for i in range(N_CORES)
    ]
    res = bass_utils.run_bass_kernel_spmd(nc, in_maps, core_ids=list(range(N_CORES)))
    out = np.concatenate([res.results[i]["out"] for i in range(N_CORES)], axis=0)
    return out.astype(np.float32, copy=False)


if __name__ == "__main__":
    rng = np.random.default_rng(0)
    sig = rng.standard_normal((B, S, L), dtype=np.float32) * 0.5
    o = kernel(signatures=sig)
    print("out", o.shape, o.dtype, float(np.abs(o).max()))


# revision 16
# speedup vs baseline: 1.5489x; 1.1521x over previous
"""Trainium2 Bass kernel for nn_ExpectedSignature.

Computes, for signatures x[B=64, S=32, L=19530] (L = sum_{k=1..6} 5^k):
  1. per-(b,s) level sums  l_k = sum_{i in level k} x_i^2
  2. c0 = 1 - phi(1 + sum_k l_k)   (phi(x) = x for x<=4 else 8 - 16/x)
  3. root u of  c0 + sum_k l_k u^k = 0  on [0,1]  (u = t^2, t = dilatation norm)
  4. out[b, i] = mean_s x[b,s,i] * t^{level(i)}

Sharding: data-parallel over batch, 8 batches per core on 8 cores.

Per-core pipeline (rows (b_local*32+s) -> 2 partition groups of 128 rows):
  - all input DMAs issue first (HWDGE stays saturated)
  - phase 1 (per group): fused square+accumulate chunks split across the
    Vector (scalar_tensor_tensor) and Scalar (activation Square) engines
  - solve (per group, Vector-only): Newton on u with an exponent-bit-trick
    6th-root seed, fused p/q Horner via scalar_tensor_tensor on [128,2]
    tiles, bit-trick sqrt + 2 Newton refines; no activation tables needed
  - phase 2 (per group): column-tiled fp32 matmuls -- 4 concurrent 32-row
    strips of the PE array write one PSUM bank [128,512] covering 2048
    output columns; stationary weights (batch-onehot/32)*t^level fuse the
    scaling and the sample mean; [128,512] staging copies then DMA out.
"""

import math
from contextlib import ExitStack

import numpy as np

import concourse.bass as bass
import concourse.bacc as bacc
import concourse.mybir as mybir
import concourse.tile as tile
from concourse import bass_utils

F32 = mybir.dt.float32
F32R = mybir.dt.float32r
I32 = mybir.dt.int32
AF = mybir.ActivationFunctionType
ALU = mybir.AluOpType
AX = mybir.AxisListType

B, S, L = 64, 32, 19530
N_CORES = 8
B_LOC = B // N_CORES          # 8 batches per core
ROWS = B_LOC * S              # 256 rows per core
N_GROUPS = 2                  # 2 partition groups of 128 rows
BPG = 4                       # batches per group
LEVEL_STARTS = [0, 5, 30, 155, 780, 3905, 19530]

MU = 0.0450465
K6 = float((1.0 - 1.0 / 6.0) * (127.0 - MU) * (1 << 23))
K2 = float(0.5 * (127.0 - MU) * (1 << 23))

CONFIG = {
    "n_newton": 4,
    "chunk": 1024,            # phase-1 compute chunk (columns)
    "dma_cols": 2700,         # target input-DMA piece size (merged chunks)
    "mm_mode": "coltile",     # "coltile" | "plain"
    "psum_cols": 512,         # PSUM tile free size (one bank)
    "psum_bufs": 4,
    "stage_bufs": 2,
    "stage_span": 4,          # psum tiles per staging tile
}

_cache = {}


def _chunk_plan(chunk):
    """Per level, split [start, end) into pieces <= chunk: (level, c0, c1)."""
    plan = []
    for k in range(6):
        c0, c1 = LEVEL_STARTS[k], LEVEL_STARTS[k + 1]
        n = c1 - c0
        pieces = max(1, math.ceil(n / chunk))
        base, rem = divmod(n, pieces)
        a = c0
        for p in range(pieces):
            sz = base + (1 if p < rem else 0)
            plan.append((k, a, a + sz))
            a += sz
        assert a == c1
    return plan


def _assign_engines(plan):
    """'v' (vector) or 's' (scalar) per chunk, balanced ~half/half."""
    eng = []
    flip = 0
    for (k, a, b) in plan:
        if k <= 2:
            eng.append("v")       # tiny levels: cheap on vector
        elif k == 3:
            eng.append("s")
        else:
            eng.append("s" if flip % 2 == 0 else "v")
            flip += 1
    return eng


def _dma_plan(plan, target):
    """Merge consecutive compute chunks into DMA pieces ~target columns."""
    pieces = []
    cur0, cur1 = None, None
    for (_, a, b) in plan:
        if cur0 is None:
            cur0, cur1 = a, b
        elif cur1 - cur0 >= target:
            pieces.append((cur0, cur1))
            cur0, cur1 = a, b
        else:
            cur1 = b
    pieces.append((cur0, cur1))
    return pieces


def _segments():
    """Column segments split at level boundaries + the 512 grid: (k, a, b)."""
    bounds = sorted(set(LEVEL_STARTS) | set(range(0, L + 1, 512)) | {L})
    segs = []
    for a, b in zip(bounds[:-1], bounds[1:]):
        k = next(i for i in range(6) if LEVEL_STARTS[i] <= a < LEVEL_STARTS[i + 1])
        segs.append((k, a, b))
    return segs


def _build_kernel(cfg):
    nc = bacc.Bacc(
        "TRN2", target_bir_lowering=False, debug=False, num_devices=N_CORES)
    x = nc.dram_tensor("x", [ROWS, L], F32, kind="ExternalInput").ap()
    wselr = nc.dram_tensor("wselr", [128, 192], F32, kind="ExternalInput").ap()
    # raw output layout: out_raw[4j+b, 5120*g + 512*i + c] =
    #   out[4g+b, 2048*i + 512*j + c]   (i = psum tile index, j = strip)
    n_pt = math.ceil(L / 2048)            # psum tiles per group (10)
    gcols = 512 * n_pt                    # raw cols per group (5120)
    out_raw = nc.dram_tensor(
        "out_raw", [16, N_GROUPS * gcols], F32, kind="ExternalOutput").ap()

    plan = _chunk_plan(cfg["chunk"])
    engines = _assign_engines(plan)
    segs = _segments()
    dma_pieces = _dma_plan(plan, cfg["dma_cols"])
    # PART layout: level k chunk j -> column NCHK*k + j (zero-padded)
    NCHK = max(sum(1 for (kk, _, _) in plan if kk == k) for k in range(6))
    part_col = {}
    ctr = [0] * 6
    for ci, (k, a, b) in enumerate(plan):
        part_col[ci] = NCHK * k + ctr[k]
        ctr[k] += 1

    with ExitStack() as ctx:
        tc = ctx.enter_context(tile.TileContext(nc))
        xg_pool = ctx.enter_context(tc.tile_pool(name="xg", bufs=1))
        cst = ctx.enter_context(tc.tile_pool(name="cst", bufs=1))
        scr_v = ctx.enter_context(tc.tile_pool(name="scr_v", bufs=2))
        scr_s = ctx.enter_context(tc.tile_pool(name="scr_s", bufs=2))
        sol = ctx.enter_context(tc.tile_pool(name="sol", bufs=1))
        psum_pool = ctx.enter_context(
            tc.tile_pool(name="psum", bufs=cfg["psum_bufs"], space="PSUM"))
        stage = ctx.enter_context(tc.tile_pool(name="stage", bufs=cfg["stage_bufs"]))

        wsel_t = cst.tile([128, 192], F32, name="wsel_t")
        nc.sync.dma_start(wsel_t[:], wselr)
        kmul = cst.tile([128, 6], F32, name="kmul")
        for j in range(6):
            nc.vector.memset(kmul[:, j:j + 1], float(6 - j))

        XG, PART, LVW, W = [], [], [], []
        for g in range(N_GROUPS):
            XG.append(xg_pool.tile([128, L], F32, name=f"xg{g}"))
            PART.append(cst.tile([128, 6 * NCHK], F32, name=f"part{g}"))
            # LVW cols: 0..5 l_k | 6 c0 | 7..12 k*l_k | 13 zero
            LVW.append(cst.tile([128, 14], F32, name=f"lvw{g}"))
            W.append(cst.tile([128, 192], F32, name=f"w{g}"))

        for g in range(N_GROUPS):
            nc.vector.memset(PART[g][:], 0.0)

        # ---- all input DMAs first (big merged pieces) ----
        for g in range(N_GROUPS):
            rows = slice(g * 128, (g + 1) * 128)
            for (a, b) in dma_pieces:
                nc.sync.dma_start(XG[g][:, a:b], x[rows, a:b])

        cp_state = [0]

        def emit_phase1(g):
            for ci, (k, a, b) in enumerate(plan):
                xt = XG[g][:, a:b]
                pc_ = part_col[ci]
                acc = PART[g][:, pc_:pc_ + 1]
                if engines[ci] == "v":
                    scr = scr_v.tile([128, cfg["chunk"]], F32, name="scrv",
                                     tag="scr_v")
                    nc.vector.scalar_tensor_tensor(
                        out=scr[:, : b - a], in0=xt, scalar=1.0, in1=xt,
                        op0=ALU.bypass, op1=ALU.mult, accum_out=acc)
                else:
                    scr = scr_s.tile([128, cfg["chunk"]], F32, name="scrs",
                                     tag="scr_s")
                    nc.scalar.activation(
                        out=scr[:, : b - a], in_=xt, func=AF.Square,
                        accum_out=acc)

        def emit_solve(g):
            # LVW cols (descending): 0..5 = l6..l1 | 6 c0 | 7..12 = 6*l6..1*l1
            # | 13 zero  -> p-scan coeffs = cols 0:7, q-scan coeffs = cols 7:14
            lvw = LVW[g]
            nc.vector.tensor_reduce(
                out=lvw[:, 0:6],
                in_=PART[g][:].rearrange("p (k j) -> p k j", j=NCHK)[:, ::-1, :],
                axis=AX.X, op=ALU.add)
            sl = sol.tile([128, 12], F32, name=f"sl{g}")
            ua = sol.tile([128, 1], F32, name=f"ua{g}")
            ub = sol.tile([128, 1], F32, name=f"ub{g}")
            pq = sol.tile([128, 2], F32, name=f"pq{g}")
            Ft = sol.tile([128, 6], F32, name=f"ft{g}")

            sumlv, nq, rnq, c0b = sl[:, 0:1], sl[:, 1:2], sl[:, 2:3], sl[:, 3:4]
            c0s, msk, dlt, rl6 = sl[:, 4:5], sl[:, 5:6], sl[:, 6:7], sl[:, 7:8]
            t1, bf, yy, tnew = sl[:, 8:9], sl[:, 9:10], sl[:, 10:11], sl[:, 11:12]

            nc.vector.tensor_reduce(out=sumlv, in_=lvw[:, 0:6], axis=AX.X,
                                    op=ALU.add)
            nc.vector.tensor_scalar(nq, sumlv, 1.0, None, ALU.add)
            nc.vector.reciprocal(rnq, nq)
            nc.vector.tensor_scalar(c0b, rnq, 16.0, -7.0, ALU.mult, ALU.add)
            nc.vector.tensor_scalar(c0s, nq, -1.0, 1.0, ALU.mult, ALU.add)
            nc.vector.tensor_scalar(msk, nq, 4.0, None, ALU.is_gt)
            nc.vector.tensor_sub(dlt, c0b, c0s)
            nc.vector.scalar_tensor_tensor(
                lvw[:, 6:7], dlt, msk[:, 0:1], c0s, op0=ALU.mult, op1=ALU.add)
            nc.vector.memset(lvw[:, 13:14], 0.0)
            nc.vector.tensor_tensor(lvw[:, 7:13], lvw[:, 0:6], kmul[:], ALU.mult)

            # seed u0 = min(1, (-c0/l6)^(1/6)) via exponent bit trick
            nc.vector.reciprocal(rl6, lvw[:, 0:1])
            nc.vector.scalar_tensor_tensor(
                t1, lvw[:, 6:7], -1.0, rl6, op0=ALU.mult, op1=ALU.mult)
            nc.vector.tensor_copy(bf, t1.bitcast(I32))       # int->float value
            nc.vector.tensor_scalar(yy, bf, 1.0 / 6.0, K6, ALU.mult, ALU.add)
            nc.vector.tensor_copy(t1.bitcast(I32), yy)       # float->int value
            nc.vector.tensor_scalar_min(ua, t1, 1.0)

            u, un = ua, ub
            scp = sol.tile([128, 7], F32, name=f"scp{g}", tag=f"scp{g}")
            scq = sol.tile([128, 7], F32, name=f"scq{g}", tag=f"scq{g}")
            for it in range(cfg["n_newton"]):
                ub_ = u[:, 0:1].broadcast_to([128, 7])
                # p = Horner(l6..l1, c0); q = u * p' = Horner(6l6..1l1, 0)
                nc.vector.tensor_tensor_scan(
                    scp[:], ub_, lvw[:, 0:7], 0.0, op0=ALU.mult, op1=ALU.add)
                nc.vector.tensor_tensor_scan(
                    scq[:], ub_, lvw[:, 7:14], 0.0, op0=ALU.mult, op1=ALU.add)
                nc.vector.tensor_sub(dlt, scq[:, 6:7], scp[:, 6:7])
                nc.vector.reciprocal(rnq, scq[:, 6:7])
                nc.vector.scalar_tensor_tensor(
                    un[:], dlt, rnq[:, 0:1], u[:], op0=ALU.mult, op1=ALU.mult)
                u, un = un, u

            # t = min(1, sqrt(u)): bit-trick seed + 2 Newton refines
            nc.vector.tensor_copy(bf, u.bitcast(I32))
            nc.vector.tensor_scalar(yy, bf, 0.5, K2, ALU.mult, ALU.add)
            nc.vector.tensor_copy(t1.bitcast(I32), yy)
            tcur = t1
            for r in range(2):
                nxt = tnew if tcur is t1 else t1
                nc.vector.reciprocal(rnq, tcur)
                nc.vector.scalar_tensor_tensor(
                    dlt, rnq, u[:, 0:1], tcur, op0=ALU.mult, op1=ALU.add)
                nc.vector.tensor_scalar(nxt, dlt, 0.5, None, ALU.mult)
                tcur = nxt
            # F = (t, u, ut, u2, u2t, u3)
            nc.vector.tensor_scalar_min(Ft[:, 0:1], tcur, 1.0)
            nc.vector.tensor_copy(Ft[:, 1:2], u[:])
            nc.vector.tensor_scalar(Ft[:, 2:4], Ft[:, 0:2], u[:, 0:1], None,
                                    ALU.mult)
            nc.vector.tensor_scalar(Ft[:, 4:6], Ft[:, 2:4], u[:, 0:1], None,
                                    ALU.mult)
            # W[:, 32k+m] = wsel[:, 32k+m] * F[:, k]  (cols m>=4 are zero)
            fb = Ft[:].unsqueeze(2).broadcast_to([128, 6, 32])
            nc.vector.tensor_tensor(W[g][:], wsel_t[:], fb, ALU.mult)

        def emit_phase2(g):
            pc = cfg["psum_cols"]
            span = 4 * pc    # out-columns covered per PSUM tile
            if cfg["mm_mode"] == "plain":
                for p0 in range(0, L, 1024):
                    p1 = min(p0 + 1024, L)
                    ps = psum_pool.tile([BPG, 1024], F32, name="ps", tag="ps")
                    for (k, a, b) in segs:
                        if a < p0 or b > p1:
                            continue
                        nc.tensor.matmul(
                            ps[:, a - p0:b - p0],
                            W[g][:, 32 * k:32 * k + BPG],
                            XG[g][:, a:b], start=True, stop=True)
                    st = stage.tile([BPG, 1024], F32, name="st", tag="st")
                    if cp_state[0] % 2 == 0:
                        nc.vector.tensor_copy(st[:, : p1 - p0], ps[:, : p1 - p0])
                    else:
                        nc.scalar.copy(st[:, : p1 - p0], ps[:, : p1 - p0])
                    cp_state[0] += 1
                    nc.sync.dma_start(
                        out[g * BPG:(g + 1) * BPG, p0:p1], st[:, : p1 - p0])
                return
            nspan = cfg["stage_span"]
            big = nspan * span      # out-columns covered per staging tile
            for big0 in range(0, L, big):
                big1 = min(big0 + big, L)
                st = stage.tile([128, nspan * pc], F32, name="st", tag="st")
                mtiles = []
                for m, tile0 in enumerate(range(big0, big1, span)):
                    tile1 = min(tile0 + span, big1)
                    ps = psum_pool.tile([128, pc], F32, name="ps", tag="ps")
                    strips = []
                    for j in range(4):
                        s0 = tile0 + j * pc
                        s1 = min(s0 + pc, tile1)
                        if s0 >= s1:
                            break
                        strips.append((j, s0, s1))
                        for (k, a, b) in segs:
                            if a < s0 or b > s1:
                                continue
                            nc.tensor.matmul(
                                ps[32 * j:32 * j + 32, a - s0:b - s0],
                                W[g][:, 32 * k:32 * (k + 1)], XG[g][:, a:b],
                                start=True, stop=True,
                                tile_position=(0, 32 * j))
                    full = len(strips) == 4 and all(
                        s1 - s0 == pc for (_, s0, s1) in strips)
                    if full:
                        if cp_state[0] % 2 == 0:
                            nc.vector.tensor_copy(
                                st[:, m * pc:(m + 1) * pc], ps[:, :])
                        else:
                            nc.scalar.copy(
                                st[:, m * pc:(m + 1) * pc], ps[:, :])
                        cp_state[0] += 1
                    else:
                        for (j, s0, s1) in strips:
                            w_ = s1 - s0
                            if cp_state[0] % 2 == 0:
                                nc.vector.tensor_copy(
                                    st[32 * j:32 * j + BPG,
                                       m * pc:m * pc + w_],
                                    ps[32 * j:32 * j + BPG, :w_])
                            else:
                                nc.scalar.copy(
                                    st[32 * j:32 * j + BPG,
                                       m * pc:m * pc + w_],
                                    ps[32 * j:32 * j + BPG, :w_])
                            cp_state[0] += 1
                    mtiles.append((m, tile0, tile1, strips))
                # fill never-written staging regions of the tail tile so
                # the raw DMA below reads fully-initialized SBUF
                nm = len(mtiles)
                tail_strips = mtiles[-1][3]
                if len(tail_strips) < 4 or any(
                        s1 - s0 < pc for (_, s0, s1) in tail_strips):
                    m_last = mtiles[-1][0]
                    base = m_last * pc
                    wmax = {j: s1 - s0 for (j, s0, s1) in tail_strips}
                    for j in range(4):
                        w_ = wmax.get(j, 0)
                        if w_ < pc:
                            nc.vector.memset(
                                st[32 * j:32 * j + 32, base + w_:base + pc],
                                0.0)
                # raw out DMAs: one per batch-row b, exact bytes
                i0 = big0 // span     # first psum-tile index in this staging tile
                W_ = nm * pc
                for j in range(4):
                    nc.sync.dma_start(
                        out_raw[4 * j:4 * j + 4,
                                g * gcols + 512 * i0:
                                g * gcols + 512 * i0 + W_],
                        st[32 * j:32 * j + 4, 0:W_])

        for g in range(N_GROUPS):
            emit_phase1(g)
            emit_solve(g)
            emit_phase2(g)

    nc.compile()
    return nc


def _get_nc():
    key = tuple(sorted((k, str(v)) for k, v in CONFIG.items()))
    if key not in _cache:
        _cache[key] = _build_kernel(CONFIG)
    return _cache[key]


def _wsel_np():
    w = np.zeros((128, 192), dtype=np.float32)
    for k in range(6):
        for j in range(BPG):
            w[j * 32:(j + 1) * 32, 32 * k + j] = 1.0 / 32.0
    return w


def assemble_out(raws):
    """raws: per-core [16, 2*5120] raw tensors -> full [B, L] output."""
    n_pt = math.ceil(L / 2048)
    gcols = 512 * n_pt
    out = np.empty((B, L), dtype=np.float32)
    for core, raw in enumerate(raws):
        for g in range(N_GROUPS):
            for b_ in range(BPG):
                row = core * B_LOC + g * BPG + b_
                for j in range(4):
                    src = raw[4 * j + b_, g * gcols:(g + 1) * gcols]
                    for i in range(n_pt):
                        a = 2048 * i + 512 * j
                        if a >= L:
                            break
                        w = min(512, L - a)
                        out[row, a:a + w] = src[512 * i:512 * i + w]
    return out


def kernel(signatures: np.ndarray, **_ignored) -> np.ndarray:
    x = np.ascontiguousarray(np.asarray(signatures), dtype=np.float32)
    assert x.shape == (B, S, L), x.shape
    nc = _get_nc()
    wsel = _wsel_np()
    in_maps = [
        {"x": np.ascontiguousarray(x[i * B_LOC:(i + 1) * B_LOC].reshape(ROWS, L)),
         "wselr": wsel}
        for i in range(N_CORES)
    ]
    res = bass_utils.run_bass_kernel_spmd(nc, in_maps, core_ids=list(range(N_CORES)))
    return assemble_out([res.results[i]["out_raw"] for i in range(N_CORES)])


if __name__ == "__main__":
    rng = np.random.default_rng(0)
    sig = rng.standard_normal((B, S, L), dtype=np.float32) * 0.5
    o = kernel(signatures=sig)
    print("out", o.shape, o.dtype, float(np.abs(o).max()))


# revision 17
# speedup vs baseline: 1.5710x; 1.0143x over previous
"""Trainium2 Bass kernel for nn_ExpectedSignature.

Computes, for signatures x[B=64, S=32, L=19530] (L = sum_{k=1..6} 5^k):
  1. per-(b,s) level sums  l_k = sum_{i in level k} x_i^2
  2. c0 = 1 - phi(1 + sum_k l_k)   (phi(x) = x for x<=4 else 8 - 16/x)
  3. root u of  c0 + sum_k l_k u^k = 0  on [0,1]  (u = t^2, t = dilatation norm)
  4. out[b, i] = mean_s x[b,s,i] * t^{level(i)}

Sharding: data-parallel over batch, 8 batches per core on 8 cores.

Per-core pipeline (rows (b_local*32+s) -> 2 partition groups of 128 rows):
  - all input DMAs issue first (HWDGE stays saturated)
  - phase 1 (per group): fused square+accumulate chunks split across the
    Vector (scalar_tensor_tensor) and Scalar (activation Square) engines
  - solve (per group, Vector-only): Newton on u with an exponent-bit-trick
    6th-root seed, fused p/q Horner via scalar_tensor_tensor on [128,2]
    tiles, bit-trick sqrt + 2 Newton refines; no activation tables needed
  - phase 2 (per group): column-tiled fp32 matmuls -- 4 concurrent 32-row
    strips of the PE array write one PSUM bank [128,512] covering 2048
    output columns; stationary weights (batch-onehot/32)*t^level fuse the
    scaling and the sample mean; [128,512] staging copies then DMA out.
"""

import math
from contextlib import ExitStack

import numpy as np

import concourse.bass as bass
import concourse.bacc as bacc
import concourse.mybir as mybir
import concourse.tile as tile
from concourse import bass_utils

F32 = mybir.dt.float32
F32R = mybir.dt.float32r
I32 = mybir.dt.int32
AF = mybir.ActivationFunctionType
ALU = mybir.AluOpType
AX = mybir.AxisListType

B, S, L = 64, 32, 19530
N_CORES = 8
B_LOC = B // N_CORES          # 8 batches per core
ROWS = B_LOC * S              # 256 rows per core
N_GROUPS = 2                  # 2 partition groups of 128 rows
BPG = 4                       # batches per group
LEVEL_STARTS = [0, 5, 30, 155, 780, 3905, 19530]

MU = 0.0450465
K6 = float((1.0 - 1.0 / 6.0) * (127.0 - MU) * (1 << 23))
K2 = float(0.5 * (127.0 - MU) * (1 << 23))

CONFIG = {
    "n_newton": 4,
    "chunk": 1024,            # phase-1 compute chunk (columns)
    "dma_cols": 2700,         # target input-DMA piece size (merged chunks)
    "mm_mode": "coltile",     # "coltile" | "plain"
    "psum_cols": 512,         # PSUM tile free size (one bank)
    "psum_bufs": 6,
    "stage_bufs": 2,
    "stage_span": 4,          # psum tiles per staging tile
}

_cache = {}


def _chunk_plan(chunk):
    """Per level, split [start, end) into pieces <= chunk: (level, c0, c1)."""
    plan = []
    for k in range(6):
        c0, c1 = LEVEL_STARTS[k], LEVEL_STARTS[k + 1]
        n = c1 - c0
        pieces = max(1, math.ceil(n / chunk))
        base, rem = divmod(n, pieces)
        a = c0
        for p in range(pieces):
            sz = base + (1 if p < rem else 0)
            plan.append((k, a, a + sz))
            a += sz
        assert a == c1
    return plan


def _assign_engines(plan):
    """'v' (vector) or 's' (scalar) per chunk, balanced ~half/half."""
    eng = []
    flip = 0
    for (k, a, b) in plan:
        if k <= 2:
            eng.append("v")       # tiny levels: cheap on vector
        elif k == 3:
            eng.append("s")
        else:
            eng.append("s" if flip % 2 == 0 else "v")
            flip += 1
    return eng


def _dma_plan(plan, target):
    """Merge consecutive compute chunks into DMA pieces ~target columns."""
    pieces = []
    cur0, cur1 = None, None
    for (_, a, b) in plan:
        if cur0 is None:
            cur0, cur1 = a, b
        elif cur1 - cur0 >= target:
            pieces.append((cur0, cur1))
            cur0, cur1 = a, b
        else:
            cur1 = b
    pieces.append((cur0, cur1))
    return pieces


def _segments():
    """Column segments split at level boundaries + the 512 grid: (k, a, b)."""
    bounds = sorted(set(LEVEL_STARTS) | set(range(0, L + 1, 512)) | {L})
    segs = []
    for a, b in zip(bounds[:-1], bounds[1:]):
        k = next(i for i in range(6) if LEVEL_STARTS[i] <= a < LEVEL_STARTS[i + 1])
        segs.append((k, a, b))
    return segs


def _build_kernel(cfg):
    nc = bacc.Bacc(
        "TRN2", target_bir_lowering=False, debug=False, num_devices=N_CORES)
    x = nc.dram_tensor("x", [ROWS, L], F32, kind="ExternalInput").ap()
    wselr = nc.dram_tensor("wselr", [128, 192], F32, kind="ExternalInput").ap()
    # raw output layout: out_raw[4j+b, 5120*g + 512*i + c] =
    #   out[4g+b, 2048*i + 512*j + c]   (i = psum tile index, j = strip)
    n_pt = math.ceil(L / 2048)            # psum tiles per group (10)
    gcols = 512 * n_pt                    # raw cols per group (5120)
    out_raw = nc.dram_tensor(
        "out_raw", [16, N_GROUPS * gcols], F32, kind="ExternalOutput").ap()

    plan = _chunk_plan(cfg["chunk"])
    engines = _assign_engines(plan)
    segs = _segments()
    dma_pieces = _dma_plan(plan, cfg["dma_cols"])
    # PART layout: level k chunk j -> column NCHK*k + j (zero-padded)
    NCHK = max(sum(1 for (kk, _, _) in plan if kk == k) for k in range(6))
    part_col = {}
    ctr = [0] * 6
    for ci, (k, a, b) in enumerate(plan):
        part_col[ci] = NCHK * k + ctr[k]
        ctr[k] += 1

    with ExitStack() as ctx:
        tc = ctx.enter_context(tile.TileContext(nc))
        xg_pool = ctx.enter_context(tc.tile_pool(name="xg", bufs=1))
        cst = ctx.enter_context(tc.tile_pool(name="cst", bufs=1))
        scr_v = ctx.enter_context(tc.tile_pool(name="scr_v", bufs=2))
        scr_s = ctx.enter_context(tc.tile_pool(name="scr_s", bufs=2))
        sol = ctx.enter_context(tc.tile_pool(name="sol", bufs=1))
        psum_pool = ctx.enter_context(
            tc.tile_pool(name="psum", bufs=cfg["psum_bufs"], space="PSUM"))
        stage = ctx.enter_context(tc.tile_pool(name="stage", bufs=cfg["stage_bufs"]))

        wsel_t = cst.tile([128, 192], F32, name="wsel_t")
        nc.sync.dma_start(wsel_t[:], wselr)
        kmul = cst.tile([128, 6], F32, name="kmul")
        for j in range(6):
            nc.vector.memset(kmul[:, j:j + 1], float(6 - j))

        XG, PART, LVW, W = [], [], [], []
        for g in range(N_GROUPS):
            XG.append(xg_pool.tile([128, L], F32, name=f"xg{g}"))
            PART.append(cst.tile([128, 6 * NCHK], F32, name=f"part{g}"))
            # LVW cols: 0..5 l_k | 6 c0 | 7..12 k*l_k | 13 zero
            LVW.append(cst.tile([128, 14], F32, name=f"lvw{g}"))
            W.append(cst.tile([128, 192], F32, name=f"w{g}"))

        for g in range(N_GROUPS):
            nc.vector.memset(PART[g][:], 0.0)

        # ---- all input DMAs first (big merged pieces) ----
        for g in range(N_GROUPS):
            rows = slice(g * 128, (g + 1) * 128)
            for (a, b) in dma_pieces:
                nc.sync.dma_start(XG[g][:, a:b], x[rows, a:b])

        cp_state = [0]

        def emit_phase1(g):
            for ci, (k, a, b) in enumerate(plan):
                xt = XG[g][:, a:b]
                pc_ = part_col[ci]
                acc = PART[g][:, pc_:pc_ + 1]
                if engines[ci] == "v":
                    scr = scr_v.tile([128, cfg["chunk"]], F32, name="scrv",
                                     tag="scr_v")
                    nc.vector.scalar_tensor_tensor(
                        out=scr[:, : b - a], in0=xt, scalar=1.0, in1=xt,
                        op0=ALU.bypass, op1=ALU.mult, accum_out=acc)
                else:
                    scr = scr_s.tile([128, cfg["chunk"]], F32, name="scrs",
                                     tag="scr_s")
                    nc.scalar.activation(
                        out=scr[:, : b - a], in_=xt, func=AF.Square,
                        accum_out=acc)

        def emit_solve(g):
            # LVW cols (descending): 0..5 = l6..l1 | 6 c0 | 7..12 = 6*l6..1*l1
            # | 13 zero  -> p-scan coeffs = cols 0:7, q-scan coeffs = cols 7:14
            lvw = LVW[g]
            nc.vector.tensor_reduce(
                out=lvw[:, 0:6],
                in_=PART[g][:].rearrange("p (k j) -> p k j", j=NCHK)[:, ::-1, :],
                axis=AX.X, op=ALU.add)
            sl = sol.tile([128, 12], F32, name=f"sl{g}")
            ua = sol.tile([128, 1], F32, name=f"ua{g}")
            ub = sol.tile([128, 1], F32, name=f"ub{g}")
            pq = sol.tile([128, 2], F32, name=f"pq{g}")
            Ft = sol.tile([128, 6], F32, name=f"ft{g}")

            sumlv, nq, rnq, c0b = sl[:, 0:1], sl[:, 1:2], sl[:, 2:3], sl[:, 3:4]
            c0s, msk, dlt, rl6 = sl[:, 4:5], sl[:, 5:6], sl[:, 6:7], sl[:, 7:8]
            t1, bf, yy, tnew = sl[:, 8:9], sl[:, 9:10], sl[:, 10:11], sl[:, 11:12]

            nc.vector.tensor_reduce(out=sumlv, in_=lvw[:, 0:6], axis=AX.X,
                                    op=ALU.add)
            nc.vector.tensor_scalar(nq, sumlv, 1.0, None, ALU.add)
            nc.vector.reciprocal(rnq, nq)
            nc.vector.tensor_scalar(c0b, rnq, 16.0, -7.0, ALU.mult, ALU.add)
            nc.vector.tensor_scalar(c0s, nq, -1.0, 1.0, ALU.mult, ALU.add)
            nc.vector.tensor_scalar(msk, nq, 4.0, None, ALU.is_gt)
            nc.vector.tensor_sub(dlt, c0b, c0s)
            nc.vector.scalar_tensor_tensor(
                lvw[:, 6:7], dlt, msk[:, 0:1], c0s, op0=ALU.mult, op1=ALU.add)
            nc.vector.memset(lvw[:, 13:14], 0.0)
            nc.vector.tensor_tensor(lvw[:, 7:13], lvw[:, 0:6], kmul[:], ALU.mult)

            # seed u0 = min(1, (-c0/l6)^(1/6)) via exponent bit trick
            nc.vector.reciprocal(rl6, lvw[:, 0:1])
            nc.vector.scalar_tensor_tensor(
                t1, lvw[:, 6:7], -1.0, rl6, op0=ALU.mult, op1=ALU.mult)
            nc.vector.tensor_copy(bf, t1.bitcast(I32))       # int->float value
            nc.vector.tensor_scalar(yy, bf, 1.0 / 6.0, K6, ALU.mult, ALU.add)
            nc.vector.tensor_copy(t1.bitcast(I32), yy)       # float->int value
            nc.vector.tensor_scalar_min(ua, t1, 1.0)

            u, un = ua, ub
            scp = sol.tile([128, 7], F32, name=f"scp{g}", tag=f"scp{g}")
            scq = sol.tile([128, 7], F32, name=f"scq{g}", tag=f"scq{g}")
            for it in range(cfg["n_newton"]):
                ub_ = u[:, 0:1].broadcast_to([128, 7])
                # p = Horner(l6..l1, c0); q = u * p' = Horner(6l6..1l1, 0)
                nc.vector.tensor_tensor_scan(
                    scp[:], ub_, lvw[:, 0:7], 0.0, op0=ALU.mult, op1=ALU.add)
                nc.vector.tensor_tensor_scan(
                    scq[:], ub_, lvw[:, 7:14], 0.0, op0=ALU.mult, op1=ALU.add)
                nc.vector.tensor_sub(dlt, scq[:, 6:7], scp[:, 6:7])
                nc.vector.reciprocal(rnq, scq[:, 6:7])
                nc.vector.scalar_tensor_tensor(
                    un[:], dlt, rnq[:, 0:1], u[:], op0=ALU.mult, op1=ALU.mult)
                u, un = un, u

            # t = min(1, sqrt(u)): bit-trick seed + 2 Newton refines
            nc.vector.tensor_copy(bf, u.bitcast(I32))
            nc.vector.tensor_scalar(yy, bf, 0.5, K2, ALU.mult, ALU.add)
            nc.vector.tensor_copy(t1.bitcast(I32), yy)
            tcur = t1
            for r in range(2):
                nxt = tnew if tcur is t1 else t1
                nc.vector.reciprocal(rnq, tcur)
                nc.vector.scalar_tensor_tensor(
                    dlt, rnq, u[:, 0:1], tcur, op0=ALU.mult, op1=ALU.add)
                nc.vector.tensor_scalar(nxt, dlt, 0.5, None, ALU.mult)
                tcur = nxt
            # F = (t, u, ut, u2, u2t, u3)
            nc.vector.tensor_scalar_min(Ft[:, 0:1], tcur, 1.0)
            nc.vector.tensor_copy(Ft[:, 1:2], u[:])
            nc.vector.tensor_scalar(Ft[:, 2:4], Ft[:, 0:2], u[:, 0:1], None,
                                    ALU.mult)
            nc.vector.tensor_scalar(Ft[:, 4:6], Ft[:, 2:4], u[:, 0:1], None,
                                    ALU.mult)
            # W[:, 32k+m] = wsel[:, 32k+m] * F[:, k]  (cols m>=4 are zero)
            fb = Ft[:].unsqueeze(2).broadcast_to([128, 6, 32])
            nc.vector.tensor_tensor(W[g][:], wsel_t[:], fb, ALU.mult)

        def emit_phase2(g):
            pc = cfg["psum_cols"]
            span = 4 * pc    # out-columns covered per PSUM tile
            if cfg["mm_mode"] == "plain":
                for p0 in range(0, L, 1024):
                    p1 = min(p0 + 1024, L)
                    ps = psum_pool.tile([BPG, 1024], F32, name="ps", tag="ps")
                    for (k, a, b) in segs:
                        if a < p0 or b > p1:
                            continue
                        nc.tensor.matmul(
                            ps[:, a - p0:b - p0],
                            W[g][:, 32 * k:32 * k + BPG],
                            XG[g][:, a:b], start=True, stop=True)
                    st = stage.tile([BPG, 1024], F32, name="st", tag="st")
                    if cp_state[0] % 2 == 0:
                        nc.vector.tensor_copy(st[:, : p1 - p0], ps[:, : p1 - p0])
                    else:
                        nc.scalar.copy(st[:, : p1 - p0], ps[:, : p1 - p0])
                    cp_state[0] += 1
                    nc.sync.dma_start(
                        out[g * BPG:(g + 1) * BPG, p0:p1], st[:, : p1 - p0])
                return
            nspan = cfg["stage_span"]
            big = nspan * span      # out-columns covered per staging tile
            for big0 in range(0, L, big):
                big1 = min(big0 + big, L)
                st = stage.tile([128, nspan * pc], F32, name="st", tag="st")
                mtiles = []
                for m, tile0 in enumerate(range(big0, big1, span)):
                    tile1 = min(tile0 + span, big1)
                    ps = psum_pool.tile([128, pc], F32, name="ps", tag="ps")
                    strips = []
                    for j in range(4):
                        s0 = tile0 + j * pc
                        s1 = min(s0 + pc, tile1)
                        if s0 >= s1:
                            break
                        strips.append((j, s0, s1))
                        for (k, a, b) in segs:
                            if a < s0 or b > s1:
                                continue
                            nc.tensor.matmul(
                                ps[32 * j:32 * j + 32, a - s0:b - s0],
                                W[g][:, 32 * k:32 * (k + 1)], XG[g][:, a:b],
                                start=True, stop=True,
                                tile_position=(0, 32 * j))
                    full = len(strips) == 4 and all(
                        s1 - s0 == pc for (_, s0, s1) in strips)
                    if full:
                        nc.scalar.copy(st[:, m * pc:(m + 1) * pc], ps[:, :])
                    else:
                        for (j, s0, s1) in strips:
                            w_ = s1 - s0
                            nc.scalar.copy(
                                st[32 * j:32 * j + BPG, m * pc:m * pc + w_],
                                ps[32 * j:32 * j + BPG, :w_])
                    mtiles.append((m, tile0, tile1, strips))
                # fill never-written staging regions of the tail tile so
                # the raw DMA below reads fully-initialized SBUF
                nm = len(mtiles)
                tail_strips = mtiles[-1][3]
                if len(tail_strips) < 4 or any(
                        s1 - s0 < pc for (_, s0, s1) in tail_strips):
                    m_last = mtiles[-1][0]
                    base = m_last * pc
                    wmax = {j: s1 - s0 for (j, s0, s1) in tail_strips}
                    for j in range(4):
                        w_ = wmax.get(j, 0)
                        if w_ < pc:
                            nc.vector.memset(
                                st[32 * j:32 * j + 32, base + w_:base + pc],
                                0.0)
                # raw out DMAs: one per batch-row b, exact bytes
                i0 = big0 // span     # first psum-tile index in this staging tile
                W_ = nm * pc
                for j in range(4):
                    nc.sync.dma_start(
                        out_raw[4 * j:4 * j + 4,
                                g * gcols + 512 * i0:
                                g * gcols + 512 * i0 + W_],
                        st[32 * j:32 * j + 4, 0:W_])

        emit_phase1(0)
        emit_solve(0)
        emit_phase1(1)
        emit_phase2(0)
        emit_solve(1)
        emit_phase2(1)

    nc.compile()
    return nc


def _get_nc():
    key = tuple(sorted((k, str(v)) for k, v in CONFIG.items()))
    if key not in _cache:
        _cache[key] = _build_kernel(CONFIG)
    return _cache[key]


def _wsel_np():
    w = np.zeros((128, 192), dtype=np.float32)
    for k in range(6):
        for j in range(BPG):
            w[j * 32:(j + 1) * 32, 32 * k + j] = 1.0 / 32.0
    return w


def assemble_out(raws):
    """raws: per-core [16, 2*5120] raw tensors -> full [B, L] output."""
    n_pt = math.ceil(L / 2048)
    gcols = 512 * n_pt
    out = np.empty((B, L), dtype=np.float32)
    for core, raw in enumerate(raws):
        for g in range(N_GROUPS):
            for b_ in range(BPG):
                row = core * B_LOC + g * BPG + b_
                for j in range(4):
                    src = raw[4 * j + b_, g * gcols:(g + 1) * gcols]
                    for i in range(n_pt):
                        a = 2048 * i + 512 * j
                        if a >= L:
                            break
                        w = min(512, L - a)
                        out[row, a:a + w] = src[512 * i:512 * i + w]
    return out


def kernel(signatures: np.ndarray, **_ignored) -> np.ndarray:
    x = np.ascontiguousarray(np.asarray(signatures), dtype=np.float32)
    assert x.shape == (B, S, L), x.shape
    nc = _get_nc()
    wsel = _wsel_np()
    in_maps = [
        {"x": np.ascontiguousarray(x[i * B_LOC:(i + 1) * B_LOC].reshape(ROWS, L)),
         "wselr": wsel}
        for i in range(N_CORES)
    ]
    res = bass_utils.run_bass_kernel_spmd(nc, in_maps, core_ids=list(range(N_CORES)))
    return assemble_out([res.results[i]["out_raw"] for i in range(N_CORES)])


if __name__ == "__main__":
    rng = np.random.default_rng(0)
    sig = rng.standard_normal((B, S, L), dtype=np.float32) * 0.5
    o = kernel(signatures=sig)
    print("out", o.shape, o.dtype, float(np.abs(o).max()))


# revision 18
# speedup vs baseline: 1.7228x; 1.0966x over previous
"""Trainium2 Bass kernel for nn_ExpectedSignature.

Computes, for signatures x[B=64, S=32, L=19530] (L = sum_{k=1..6} 5^k):
  1. per-(b,s) level sums  l_k = sum_{i in level k} x_i^2
  2. c0 = 1 - phi(1 + sum_k l_k)   (phi(x) = x for x<=4 else 8 - 16/x)
  3. root u of  c0 + sum_k l_k u^k = 0  on [0,1]  (u = t^2, t = dilatation norm)
  4. out[b, i] = mean_s x[b,s,i] * t^{level(i)}

Sharding: data-parallel over batch, 8 batches per core on 8 cores.

Per-core pipeline (rows (b_local*32+s) -> 2 partition groups of 128 rows):
  - all input DMAs issue first (HWDGE stays saturated)
  - phase 1 (per group): fused square+accumulate chunks split across the
    Vector (scalar_tensor_tensor) and Scalar (activation Square) engines
  - solve (per group, Vector-only): Newton on u with an exponent-bit-trick
    6th-root seed, fused p/q Horner via scalar_tensor_tensor on [128,2]
    tiles, bit-trick sqrt + 2 Newton refines; no activation tables needed
  - phase 2 (per group): column-tiled fp32 matmuls -- 4 concurrent 32-row
    strips of the PE array write one PSUM bank [128,512] covering 2048
    output columns; stationary weights (batch-onehot/32)*t^level fuse the
    scaling and the sample mean; [128,512] staging copies then DMA out.
"""

import math
from contextlib import ExitStack

import numpy as np

import concourse.bass as bass
import concourse.bacc as bacc
import concourse.mybir as mybir
import concourse.tile as tile
from concourse import bass_utils

F32 = mybir.dt.float32
F32R = mybir.dt.float32r
I32 = mybir.dt.int32
AF = mybir.ActivationFunctionType
ALU = mybir.AluOpType
AX = mybir.AxisListType

B, S, L = 64, 32, 19530
N_CORES = 8
B_LOC = B // N_CORES          # 8 batches per core
ROWS = B_LOC * S              # 256 rows per core
N_GROUPS = 2                  # 2 partition groups of 128 rows
BPG = 4                       # batches per group
LEVEL_STARTS = [0, 5, 30, 155, 780, 3905, 19530]

MU = 0.0450465
K6 = float((1.0 - 1.0 / 6.0) * (127.0 - MU) * (1 << 23))
K2 = float(0.5 * (127.0 - MU) * (1 << 23))

CONFIG = {
    "n_newton": 4,
    "chunk": 1024,            # phase-1 compute chunk (columns)
    "dma_cols": 2700,         # target input-DMA piece size (merged chunks)
    "mm_mode": "coltile",     # "coltile" | "plain"
    "psum_cols": 512,         # PSUM tile free size (one bank)
    "psum_bufs": 8,
    "stage_bufs": 2,
    "stage_span": 4,          # psum tiles per staging tile
}

_cache = {}


def _chunk_plan(chunk):
    """Per level, split [start, end) into pieces <= chunk: (level, c0, c1)."""
    plan = []
    for k in range(6):
        c0, c1 = LEVEL_STARTS[k], LEVEL_STARTS[k + 1]
        n = c1 - c0
        pieces = max(1, math.ceil(n / chunk))
        base, rem = divmod(n, pieces)
        a = c0
        for p in range(pieces):
            sz = base + (1 if p < rem else 0)
            plan.append((k, a, a + sz))
            a += sz
        assert a == c1
    return plan


def _assign_engines(plan):
    """'v' (vector) or 's' (scalar) per chunk, balanced ~half/half."""
    eng = []
    flip = 0
    for (k, a, b) in plan:
        if k <= 2:
            eng.append("v")       # tiny levels: cheap on vector
        elif k == 3:
            eng.append("s")
        else:
            eng.append("s" if flip % 2 == 0 else "v")
            flip += 1
    return eng


def _dma_plan(plan, target):
    """Merge consecutive compute chunks into DMA pieces ~target columns."""
    pieces = []
    cur0, cur1 = None, None
    for (_, a, b) in plan:
        if cur0 is None:
            cur0, cur1 = a, b
        elif cur1 - cur0 >= target:
            pieces.append((cur0, cur1))
            cur0, cur1 = a, b
        else:
            cur1 = b
    pieces.append((cur0, cur1))
    return pieces


def _segments():
    """Column segments split at level boundaries + the 512 grid: (k, a, b)."""
    bounds = sorted(set(LEVEL_STARTS) | set(range(0, L + 1, 512)) | {L})
    segs = []
    for a, b in zip(bounds[:-1], bounds[1:]):
        k = next(i for i in range(6) if LEVEL_STARTS[i] <= a < LEVEL_STARTS[i + 1])
        segs.append((k, a, b))
    return segs


def _build_kernel(cfg):
    nc = bacc.Bacc(
        "TRN2", target_bir_lowering=False, debug=False, num_devices=N_CORES)
    x = nc.dram_tensor("x", [ROWS, L], F32, kind="ExternalInput").ap()
    wselr = nc.dram_tensor("wselr", [128, 192], F32, kind="ExternalInput").ap()
    # raw output layout: out_raw[4j+b, 5120*g + 512*i + c] =
    #   out[4g+b, 2048*i + 512*j + c]   (i = psum tile index, j = strip)
    n_pt = math.ceil(L / 2048)            # psum tiles per group (10)
    gcols = 512 * n_pt                    # raw cols per group (5120)
    out_raw = nc.dram_tensor(
        "out_raw", [16, N_GROUPS * gcols], F32, kind="ExternalOutput").ap()

    plan = _chunk_plan(cfg["chunk"])
    engines = _assign_engines(plan)
    segs = _segments()
    dma_pieces = _dma_plan(plan, cfg["dma_cols"])
    # PART layout: level k chunk j -> column NCHK*k + j (zero-padded)
    NCHK = max(sum(1 for (kk, _, _) in plan if kk == k) for k in range(6))
    part_col = {}
    ctr = [0] * 6
    for ci, (k, a, b) in enumerate(plan):
        part_col[ci] = NCHK * k + ctr[k]
        ctr[k] += 1

    with ExitStack() as ctx:
        tc = ctx.enter_context(tile.TileContext(nc))
        xg_pool = ctx.enter_context(tc.tile_pool(name="xg", bufs=1))
        cst = ctx.enter_context(tc.tile_pool(name="cst", bufs=1))
        scr_v = ctx.enter_context(tc.tile_pool(name="scr_v", bufs=2))
        scr_s = ctx.enter_context(tc.tile_pool(name="scr_s", bufs=2))
        sol = ctx.enter_context(tc.tile_pool(name="sol", bufs=1))
        psum_pool = ctx.enter_context(
            tc.tile_pool(name="psum", bufs=cfg["psum_bufs"], space="PSUM"))
        stage = ctx.enter_context(tc.tile_pool(name="stage", bufs=cfg["stage_bufs"]))

        wsel_t = cst.tile([128, 192], F32, name="wsel_t")
        nc.sync.dma_start(wsel_t[:], wselr)
        kmul = cst.tile([128, 6], F32, name="kmul")
        for j in range(6):
            nc.vector.memset(kmul[:, j:j + 1], float(6 - j))

        XG, PART, LVW, W = [], [], [], []
        for g in range(N_GROUPS):
            XG.append(xg_pool.tile([128, L], F32, name=f"xg{g}"))
            PART.append(cst.tile([128, 6 * NCHK], F32, name=f"part{g}"))
            # LVW cols: 0..5 l_k | 6 c0 | 7..12 k*l_k | 13 zero
            LVW.append(cst.tile([128, 14], F32, name=f"lvw{g}"))
            W.append(cst.tile([128, 192], F32, name=f"w{g}"))

        for g in range(N_GROUPS):
            nc.vector.memset(PART[g][:], 0.0)

        # ---- all input DMAs first (big merged pieces) ----
        for g in range(N_GROUPS):
            rows = slice(g * 128, (g + 1) * 128)
            for (a, b) in dma_pieces:
                nc.sync.dma_start(XG[g][:, a:b], x[rows, a:b])

        cp_state = [0]

        def emit_phase1(g):
            for ci, (k, a, b) in enumerate(plan):
                xt = XG[g][:, a:b]
                pc_ = part_col[ci]
                acc = PART[g][:, pc_:pc_ + 1]
                if engines[ci] == "v":
                    scr = scr_v.tile([128, cfg["chunk"]], F32, name="scrv",
                                     tag="scr_v")
                    nc.vector.scalar_tensor_tensor(
                        out=scr[:, : b - a], in0=xt, scalar=1.0, in1=xt,
                        op0=ALU.bypass, op1=ALU.mult, accum_out=acc)
                else:
                    scr = scr_s.tile([128, cfg["chunk"]], F32, name="scrs",
                                     tag="scr_s")
                    nc.scalar.activation(
                        out=scr[:, : b - a], in_=xt, func=AF.Square,
                        accum_out=acc)

        def emit_solve(g):
            # LVW cols (descending): 0..5 = l6..l1 | 6 c0 | 7..12 = 6*l6..1*l1
            # | 13 zero  -> p-scan coeffs = cols 0:7, q-scan coeffs = cols 7:14
            lvw = LVW[g]
            nc.vector.tensor_reduce(
                out=lvw[:, 0:6],
                in_=PART[g][:].rearrange("p (k j) -> p k j", j=NCHK)[:, ::-1, :],
                axis=AX.X, op=ALU.add)
            sl = sol.tile([128, 12], F32, name=f"sl{g}")
            ua = sol.tile([128, 1], F32, name=f"ua{g}")
            ub = sol.tile([128, 1], F32, name=f"ub{g}")
            pq = sol.tile([128, 2], F32, name=f"pq{g}")
            Ft = sol.tile([128, 6], F32, name=f"ft{g}")

            sumlv, nq, rnq, c0b = sl[:, 0:1], sl[:, 1:2], sl[:, 2:3], sl[:, 3:4]
            c0s, msk, dlt, rl6 = sl[:, 4:5], sl[:, 5:6], sl[:, 6:7], sl[:, 7:8]
            t1, bf, yy, tnew = sl[:, 8:9], sl[:, 9:10], sl[:, 10:11], sl[:, 11:12]

            nc.vector.tensor_reduce(out=sumlv, in_=lvw[:, 0:6], axis=AX.X,
                                    op=ALU.add)
            nc.vector.tensor_scalar(nq, sumlv, 1.0, None, ALU.add)
            nc.vector.reciprocal(rnq, nq)
            nc.vector.tensor_scalar(c0b, rnq, 16.0, -7.0, ALU.mult, ALU.add)
            nc.vector.tensor_scalar(c0s, nq, -1.0, 1.0, ALU.mult, ALU.add)
            nc.vector.tensor_scalar(msk, nq, 4.0, None, ALU.is_gt)
            nc.vector.tensor_sub(dlt, c0b, c0s)
            nc.vector.scalar_tensor_tensor(
                lvw[:, 6:7], dlt, msk[:, 0:1], c0s, op0=ALU.mult, op1=ALU.add)
            nc.vector.memset(lvw[:, 13:14], 0.0)
            nc.vector.tensor_tensor(lvw[:, 7:13], lvw[:, 0:6], kmul[:], ALU.mult)

            # seed u0 = min(1, (-c0/l6)^(1/6)) via exponent bit trick
            nc.vector.reciprocal(rl6, lvw[:, 0:1])
            nc.vector.scalar_tensor_tensor(
                t1, lvw[:, 6:7], -1.0, rl6, op0=ALU.mult, op1=ALU.mult)
            nc.vector.tensor_copy(bf, t1.bitcast(I32))       # int->float value
            nc.vector.tensor_scalar(yy, bf, 1.0 / 6.0, K6, ALU.mult, ALU.add)
            nc.vector.tensor_copy(t1.bitcast(I32), yy)       # float->int value
            nc.vector.tensor_scalar_min(ua, t1, 1.0)

            u, un = ua, ub
            scp = sol.tile([128, 7], F32, name=f"scp{g}", tag=f"scp{g}")
            scq = sol.tile([128, 7], F32, name=f"scq{g}", tag=f"scq{g}")
            for it in range(cfg["n_newton"]):
                ub_ = u[:, 0:1].broadcast_to([128, 7])
                # p = Horner(l6..l1, c0); q = u * p' = Horner(6l6..1l1, 0)
                nc.vector.tensor_tensor_scan(
                    scp[:], ub_, lvw[:, 0:7], 0.0, op0=ALU.mult, op1=ALU.add)
                nc.vector.tensor_tensor_scan(
                    scq[:], ub_, lvw[:, 7:14], 0.0, op0=ALU.mult, op1=ALU.add)
                nc.vector.tensor_sub(dlt, scq[:, 6:7], scp[:, 6:7])
                nc.vector.reciprocal(rnq, scq[:, 6:7])
                nc.vector.scalar_tensor_tensor(
                    un[:], dlt, rnq[:, 0:1], u[:], op0=ALU.mult, op1=ALU.mult)
                u, un = un, u

            # t = min(1, sqrt(u)): bit-trick seed + 2 Newton refines
            nc.vector.tensor_copy(bf, u.bitcast(I32))
            nc.vector.tensor_scalar(yy, bf, 0.5, K2, ALU.mult, ALU.add)
            nc.vector.tensor_copy(t1.bitcast(I32), yy)
            tcur = t1
            for r in range(2):
                nxt = tnew if tcur is t1 else t1
                nc.vector.reciprocal(rnq, tcur)
                nc.vector.scalar_tensor_tensor(
                    dlt, rnq, u[:, 0:1], tcur, op0=ALU.mult, op1=ALU.add)
                nc.vector.tensor_scalar(nxt, dlt, 0.5, None, ALU.mult)
                tcur = nxt
            # F = (t, u, ut, u2, u2t, u3)
            nc.vector.tensor_scalar_min(Ft[:, 0:1], tcur, 1.0)
            nc.vector.tensor_copy(Ft[:, 1:2], u[:])
            nc.vector.tensor_scalar(Ft[:, 2:4], Ft[:, 0:2], u[:, 0:1], None,
                                    ALU.mult)
            nc.vector.tensor_scalar(Ft[:, 4:6], Ft[:, 2:4], u[:, 0:1], None,
                                    ALU.mult)
            # W[:, 32k+m] = wsel[:, 32k+m] * F[:, k]  (cols m>=4 are zero)
            fb = Ft[:].unsqueeze(2).broadcast_to([128, 6, 32])
            nc.vector.tensor_tensor(W[g][:], wsel_t[:], fb, ALU.mult)

        def emit_phase2(g):
            pc = cfg["psum_cols"]
            span = 4 * pc    # out-columns covered per PSUM tile
            if cfg["mm_mode"] == "plain":
                for p0 in range(0, L, 1024):
                    p1 = min(p0 + 1024, L)
                    ps = psum_pool.tile([BPG, 1024], F32, name="ps", tag="ps")
                    for (k, a, b) in segs:
                        if a < p0 or b > p1:
                            continue
                        nc.tensor.matmul(
                            ps[:, a - p0:b - p0],
                            W[g][:, 32 * k:32 * k + BPG],
                            XG[g][:, a:b], start=True, stop=True)
                    st = stage.tile([BPG, 1024], F32, name="st", tag="st")
                    if cp_state[0] % 2 == 0:
                        nc.vector.tensor_copy(st[:, : p1 - p0], ps[:, : p1 - p0])
                    else:
                        nc.scalar.copy(st[:, : p1 - p0], ps[:, : p1 - p0])
                    cp_state[0] += 1
                    nc.sync.dma_start(
                        out[g * BPG:(g + 1) * BPG, p0:p1], st[:, : p1 - p0])
                return
            nspan = cfg["stage_span"]
            big = nspan * span      # out-columns covered per staging tile
            for big0 in range(0, L, big):
                big1 = min(big0 + big, L)
                st = stage.tile([128, nspan * pc], F32, name="st", tag="st")
                mtiles = []
                for m, tile0 in enumerate(range(big0, big1, span)):
                    tile1 = min(tile0 + span, big1)
                    ps = psum_pool.tile([128, pc], F32, name="ps", tag="ps")
                    strips = []
                    for j in range(4):
                        s0 = tile0 + j * pc
                        s1 = min(s0 + pc, tile1)
                        if s0 >= s1:
                            break
                        strips.append((j, s0, s1))
                        for (k, a, b) in segs:
                            if a < s0 or b > s1:
                                continue
                            nc.tensor.matmul(
                                ps[32 * j:32 * j + 32, a - s0:b - s0],
                                W[g][:, 32 * k:32 * (k + 1)], XG[g][:, a:b],
                                start=True, stop=True,
                                tile_position=(0, 32 * j))
                    full = len(strips) == 4 and all(
                        s1 - s0 == pc for (_, s0, s1) in strips)
                    if full:
                        nc.scalar.copy(st[:, m * pc:(m + 1) * pc], ps[:, :])
                    else:
                        for (j, s0, s1) in strips:
                            w_ = s1 - s0
                            nc.scalar.copy(
                                st[32 * j:32 * j + BPG, m * pc:m * pc + w_],
                                ps[32 * j:32 * j + BPG, :w_])
                    mtiles.append((m, tile0, tile1, strips))
                # fill never-written staging regions of the tail tile so
                # the raw DMA below reads fully-initialized SBUF
                nm = len(mtiles)
                tail_strips = mtiles[-1][3]
                if len(tail_strips) < 4 or any(
                        s1 - s0 < pc for (_, s0, s1) in tail_strips):
                    m_last = mtiles[-1][0]
                    base = m_last * pc
                    wmax = {j: s1 - s0 for (j, s0, s1) in tail_strips}
                    for j in range(4):
                        w_ = wmax.get(j, 0)
                        if w_ < pc:
                            nc.vector.memset(
                                st[32 * j:32 * j + 32, base + w_:base + pc],
                                0.0)
                # raw out DMAs: one per batch-row b, exact bytes
                i0 = big0 // span     # first psum-tile index in this staging tile
                W_ = nm * pc
                for j in range(4):
                    nc.sync.dma_start(
                        out_raw[4 * j:4 * j + 4,
                                g * gcols + 512 * i0:
                                g * gcols + 512 * i0 + W_],
                        st[32 * j:32 * j + 4, 0:W_])

        emit_phase1(0)
        emit_solve(0)
        emit_phase1(1)
        emit_phase2(0)
        emit_solve(1)
        emit_phase2(1)

    nc.compile()
    return nc


def _get_nc():
    key = tuple(sorted((k, str(v)) for k, v in CONFIG.items()))
    if key not in _cache:
        _cache[key] = _build_kernel(CONFIG)
    return _cache[key]


def _wsel_np():
    w = np.zeros((128, 192), dtype=np.float32)
    for k in range(6):
        for j in range(BPG):
            w[j * 32:(j + 1) * 32, 32 * k + j] = 1.0 / 32.0
    return w


def assemble_out(raws):
    """raws: per-core [16, 2*5120] raw tensors -> full [B, L] output."""
    n_pt = math.ceil(L / 2048)
    gcols = 512 * n_pt
    out = np.empty((B, L), dtype=np.float32)
    for core, raw in enumerate(raws):
        for g in range(N_GROUPS):
            for b_ in range(BPG):
                row = core * B_LOC + g * BPG + b_
                for j in range(4):
                    src = raw[4 * j + b_, g * gcols:(g + 1) * gcols]
                    for i in range(n_pt):
                        a = 2048 * i + 512 * j
                        if a >= L:
                            break
                        w = min(512, L - a)
                        out[row, a:a + w] = src[512 * i:512 * i + w]
    return out


def kernel(signatures: np.ndarray, **_ignored) -> np.ndarray:
    x = np.ascontiguousarray(np.asarray(signatures), dtype=np.float32)
    assert x.shape == (B, S, L), x.shape
    nc = _get_nc()
    wsel = _wsel_np()
    in_maps = [
        {"x": np.ascontiguousarray(x[i * B_LOC:(i + 1) * B_LOC].reshape(ROWS, L)),
         "wselr": wsel}
        for i in range(N_CORES)
    ]
    res = bass_utils.run_bass_kernel_spmd(nc, in_maps, core_ids=list(range(N_CORES)))
    return assemble_out([res.results[i]["out_raw"] for i in range(N_CORES)])


if __name__ == "__main__":
    rng = np.random.default_rng(0)
    sig = rng.standard_normal((B, S, L), dtype=np.float32) * 0.5
    o = kernel(signatures=sig)
    print("out", o.shape, o.dtype, float(np.abs(o).max()))
